# revision 28
# baseline (speedup 1.0000x reference)
"""DeepBSDE forward-loss kernel for Trainium2 (8 NeuronCores, data-parallel).

Math (per sample b, 50 steps, dt=0.02):
    x_n = [t_n, y_n]                       (4 features)
    z_n = MLP_z(x_n)   (4->64->64->3, relu)
    q_n = MLP_q(x_n)   (4->64->64->1, relu)
    y_{n+1} = (1-dt) y_n + dt q_n + (0.2 + 0.1 tanh(y_n)) * sqrt(dt) * dW_n
    Y_final = Y0 - 0.5 dt sum_n q_n^2 + sum_n z_n . (sqrt(dt) dW_n)
    out = mean_b (Y_final - |y_final|^2)^2

Device layout (per core, B_loc = 16384 = 32 chunks x 512):
    every per-sample state lives in a [128, 512] SBUF tile:
      partition k        (k in 0..31)   : q-slot of chunk k
      partition 32+32i+k (i in 0..2)    : vector component i of chunk k
      free c                            : sample index b = k*512 + c
    The two MLPs are fused: hidden = [q-hidden(64) ; z-hidden(64)] = 128.
    Per step the PE streams: L1 (K=3), L2 (K=128), L3 (per-chunk sparse
    [128,128] stationaries accumulated into ONE packed psum bank), plus one
    broadcast matmul that replicates dt*q into the 3 component quarters.
"""

import sys
import os

for _p in ("/opt/trn_rl_repo", "/root/.axon_site/_ro/trn_rl_repo"):
    if os.path.isdir(_p) and _p not in sys.path:
        sys.path.insert(0, _p)

import numpy as np

DT = 0.02
SQRT_DT = float(np.sqrt(np.float32(DT)))
N_STEPS = 50
BATCH = 131072
DIM = 3
N_CORES = 8
B_LOC = BATCH // N_CORES          # 16384
CHUNKS = 32
FREE = B_LOC // CHUNKS            # 512

# dtype knobs.
#  - L1/BB/TT matmuls read fp32 state; run them as float32r (same 4-byte
#    storage, 1 cycle/row on the PE at moving size >= 256 vs 4 for fp32).
#  - hidden activations h1/h2 and the L2/L3 weights run in bf16: same PE
#    rate as f32r but half the ACT/DVE evacuation cost and half the
#    weight-load traffic.
MM_HID_F32 = os.environ.get("BSDE_HID_F32", "0") == "1"

# how many h2 evacuations run on the scalar (ACT) engine instead of DVE,
# to balance the two engines' per-step load.
ACT_H2 = int(os.environ.get("BSDE_ACT_H2", "2"))

# offload the final-reduction accumulation ops (sqA product, accA/accP adds,
# p6 product) to the otherwise-idle GPSIMD engine.
GP_ACC = os.environ.get("BSDE_GP_ACC", "1") == "1"

# L1 matmuls in bf16 (stationary + a per-step bf16 copy of y): halves the
# f32r weight-load time on the PE at a tiny precision cost.
L1_BF16 = os.environ.get("BSDE_L1_BF16", "0") == "1"

# split the out3s evacuation: q rows via ACT feed a K=32 broadcast matmul so
# the z-row evacuation leaves the step-boundary critical path.
SPLIT_TAIL = os.environ.get("BSDE_SPLIT_TAIL", "0") == "1"

# emit PE work pair-adjacent (L1,L1,L2,L2,L3,L3 per chunk pair) so the two
# W2 loads sit back-to-back for weight-load pull-ahead.
PAIR_EMIT = os.environ.get("BSDE_PAIR_EMIT", "0") == "1"

# deeper activation rings to absorb evacuation jitter (SBUF has headroom)
RING_H = os.environ.get("BSDE_RING_H", "1") == "1"

# give ps2 a 4th bank by allocating pq from ps3's ring (p3's bank is free by
# the time the broadcast runs; the WAR dep is exactly the required ordering)
PS2_4 = os.environ.get("BSDE_PS2_4", "1") == "1"

# If set (by the timing harness), the device loop runs this many steps while
# all I/O shapes stay identical — lets wall-clock differencing isolate the
# per-step device time from RPC/transfer overhead.
LOOP_STEPS = None


def _np_f32(x):
    return np.ascontiguousarray(np.asarray(x, dtype=np.float32))


def prep_host(inputs):
    """Build all device-side arrays (numpy fp32) from the raw problem inputs."""
    i = {k: _np_f32(v) for k, v in inputs.items()}
    qW1, qb1 = i["qW1"], i["qb1"]
    qW2, qb2 = i["qW2"], i["qb2"]
    qW3, qb3 = i["qW3"], i["qb3"]
    zW1, zb1 = i["zW1"], i["zb1"]
    zW2, zb2 = i["zW2"], i["zb2"]
    zW3, zb3 = i["zW3"], i["zb3"]
    y0 = i["y0"]
    Y0 = float(i["Y0"].reshape(-1)[0])
    dW = i["dW"]

    W1cat = np.concatenate([qW1, zW1], axis=1)          # [4, 128]
    b1cat = np.concatenate([qb1, zb1])                  # [128]

    # L1 reads the packed y state directly: per-chunk sparse stationaries.
    # chunk k's component i lives at partition 32 + 32*i + k.
    W1S = np.zeros((CHUNKS, 128, 128), np.float32)
    for k in range(CHUNKS):
        for comp in range(3):
            W1S[k, 32 + 32 * comp + k, :] = W1cat[1 + comp, :]
    W1S = np.ascontiguousarray(W1S.transpose(1, 0, 2).reshape(128, CHUNKS * 128))

    # per-step bias for the L1 relu evacuation: c_n = t_n * W1cat[0] + b1cat
    ts = (np.arange(N_STEPS, dtype=np.float32) * np.float32(DT))
    CB = (ts[None, :] * W1cat[0][:, None] + b1cat[:, None]).astype(np.float32)  # [128, 50]

    W2 = np.zeros((128, 128), np.float32)
    W2[0:64, 0:64] = qW2
    W2[64:128, 64:128] = zW2
    B2 = b1cat * 0.0
    B2 = np.concatenate([qb2, zb2]).astype(np.float32).reshape(128, 1)

    # L3: per-chunk sparse stationaries [128, 32*128]
    W3S = np.zeros((CHUNKS, 128, 128), np.float32)
    for k in range(CHUNKS):
        W3S[k, 0:64, k] = DT * qW3[:, 0]
        for comp in range(3):
            W3S[k, 64:128, 32 + 32 * comp + k] = SQRT_DT * zW3[:, comp]
    W3S = np.ascontiguousarray(W3S.transpose(1, 0, 2).reshape(128, CHUNKS * 128))

    B3 = np.zeros((128, 1), np.float32)
    B3[0:32, 0] = DT * qb3[0]
    for comp in range(3):
        B3[32 + 32 * comp:64 + 32 * comp, 0] = SQRT_DT * zb3[comp]

    # broadcast matmul: qrep[32+32i+k] = out3s[k]
    BB = np.zeros((128, 128), np.float32)
    for k in range(CHUNKS):
        for comp in range(3):
            BB[k, 32 + 32 * comp + k] = 1.0

    # final reduction: col j sums the 3 components of chunk j
    TT = np.zeros((128, 32), np.float32)
    for j in range(CHUNKS):
        for comp in range(3):
            TT[32 + 32 * comp + j, j] = 1.0

    # initial y state, broadcast to full [128, 512] (q-slot rows zero)
    YINIT = np.zeros((128, FREE), np.float32)
    for comp in range(3):
        YINIT[32 + 32 * comp:64 + 32 * comp, :] = y0[comp]

    # per-core dW, transposed to [steps, comp, b_loc]; row block 0 (the q-slot
    # partitions) is zeros so a single full-tile DMA initializes everything
    dWt_cores = []
    for r in range(N_CORES):
        sl = dW[:N_STEPS, r * B_LOC:(r + 1) * B_LOC, :]     # [N_STEPS, B_loc, 3]
        t = np.zeros((N_STEPS, 4 * B_LOC), np.float32)
        t[:, B_LOC:] = sl.transpose(0, 2, 1).reshape(N_STEPS, 3 * B_LOC)
        dWt_cores.append(t)

    return dict(
        W1S=W1S, CB=CB, W2=W2, B2=B2, W3S=W3S, B3=B3, BB=BB, TT=TT,
        YINIT=YINIT, dWt_cores=dWt_cores, Y0=Y0,
    )


def _split_sync_waits(bir: dict) -> dict:
    """Walrus in this toolchain accepts only ~1 sync wait per instruction.
    Hoist extra waits onto standalone EventSemaphore instructions inserted
    just before, on the same engine (waits-only, so semantics unchanged)."""
    n = 0
    for fn in bir.get("functions", []):
        for bb in fn.get("blocks", []):
            out = []
            for ins in bb.get("instructions", []):
                si = ins.get("sync_info")
                waits = (si or {}).get("on_wait") or []
                if len(waits) > 1:
                    for w in waits[:-1]:
                        n += 1
                        out.append({
                            "engine": ins["engine"],
                            "ins": [],
                            "outs": [],
                            "name": f"bsdewait{n}_{ins['name']}",
                            "opcode": "EventSemaphore",
                            "debug": ins.get("debug", 0),
                            "sync_info": {"on_update": [], "on_wait": [w]},
                        })
                    si["on_wait"] = [waits[-1]]
                out.append(ins)
            bb["instructions"] = out
    return bir


def _install_ldw_opt():
    """walrus is invoked with --enable-ldw-opt=false; flip it on so repeated/
    adjacent stationary loads are optimized (gated by BSDE_LDW_OPT)."""
    from concourse import bass_utils
    if os.environ.get("BSDE_LDW_OPT", "0") != "1":
        return
    if getattr(bass_utils, "_bsde_ldwopt_installed", False):
        return
    orig = bass_utils.run_command

    def wrapped(cmd, **kw):
        if isinstance(cmd, list):
            cmd = ["--enable-ldw-opt=true" if c == "--enable-ldw-opt=false" else c
                   for c in cmd]
        return orig(cmd, **kw)

    bass_utils.run_command = wrapped
    bass_utils._bsde_ldwopt_installed = True


def _install_wait_splitter():
    import json as _json
    from concourse import bass2jax, bass_utils
    if getattr(bass_utils, "_bsde_split_installed", False):
        return
    orig = bass_utils.compile_bir_kernel

    def wrapped(bir_json, tmpdir, neff_name="file.neff"):
        bir = _json.loads(bir_json)
        _split_sync_waits(bir)
        return orig(_json.dumps(bir).encode(), tmpdir, neff_name)

    bass_utils.compile_bir_kernel = wrapped
    bass2jax.compile_bir_kernel = wrapped
    bass_utils._bsde_split_installed = True


def build_program():
    """Build the Bass program (same for all cores). Returns (nc, meta)."""
    from concourse import bass, mybir, tile

    f32 = mybir.dt.float32
    f32r = mybir.dt.float32r
    hdt = f32 if MM_HID_F32 else mybir.dt.bfloat16
    Alu = mybir.AluOpType
    Act = mybir.ActivationFunctionType

    def R(ap):
        # reinterpret fp32 data as float32r for full-rate PE streaming
        return ap.bitcast(f32r)

    def H(ap):
        # hidden-path operand: native bf16, or f32->f32r bitcast in fallback
        return R(ap) if hdt == f32 else ap

    nc = bass.Bass("TRN2", target_bir_lowering=False, debug=False)

    # --- dram I/O ---
    d_dWt = nc.dram_tensor("dWt", [N_STEPS, 4 * B_LOC], f32, kind="ExternalInput").ap()
    d_W1S = nc.dram_tensor("W1S", [128, CHUNKS * 128], f32r, kind="ExternalInput").ap()
    d_CB = nc.dram_tensor("CB", [128, N_STEPS], f32, kind="ExternalInput").ap()
    d_W2 = nc.dram_tensor("W2", [128, 128], f32, kind="ExternalInput").ap()
    d_B2 = nc.dram_tensor("B2", [128, 1], f32, kind="ExternalInput").ap()
    d_W3S = nc.dram_tensor("W3S", [128, CHUNKS * 128], f32, kind="ExternalInput").ap()
    d_B3 = nc.dram_tensor("B3", [128, 1], f32, kind="ExternalInput").ap()
    d_BB = nc.dram_tensor("BB", [128, 128], f32r, kind="ExternalInput").ap()
    d_TT = nc.dram_tensor("TT", [128, 32], f32r, kind="ExternalInput").ap()
    d_YI = nc.dram_tensor("YINIT", [128, FREE], f32r, kind="ExternalInput").ap()
    d_SC = nc.dram_tensor("SCAL", [4, 1], f32, kind="ExternalInput").ap()  # [Y0; -0.5/dt; a; b]
    d_res = nc.dram_tensor("res", [32, 1], f32, kind="ExternalOutput").ap()

    a_coef = 0.1 * SQRT_DT
    b_coef = 0.2 * SQRT_DT

    with tile.TileContext(nc) as tc:
        with (
            tc.tile_pool(name="consts", bufs=1) as consts,
            tc.tile_pool(name="state", bufs=1) as state,
            tc.tile_pool(name="h1p", bufs=(6 if RING_H else 3)) as h1pool,
            tc.tile_pool(name="h2p", bufs=(8 if RING_H else 4)) as h2pool,
            tc.tile_pool(name="tmp", bufs=1) as tmp,
            tc.tile_pool(name="dwp", bufs=8) as dwp,
            tc.tile_pool(name="ps1", bufs=3, space="PSUM") as ps1,
            tc.tile_pool(name="ps2", bufs=(4 if PS2_4 else 3), space="PSUM") as ps2,
            tc.tile_pool(name="ps3", bufs=1, space="PSUM") as ps3,
            tc.tile_pool(name="psq", bufs=1, space="PSUM") as psq,
        ):
            # ---- load constants into SBUF ----
            _dma_eng = [nc.sync, nc.scalar, nc.gpsimd]
            _dma_i = [0]

            def load_const(name, dram_ap, shape, dt_):
                t = consts.tile(shape, dt_, tag=name)
                eng = _dma_eng[_dma_i[0] % len(_dma_eng)]
                _dma_i[0] += 1
                eng.dma_start(t[:], dram_ap)
                return t

            W1S = load_const("W1S", d_W1S, [128, CHUNKS * 128], f32r)
            CB = load_const("CB", d_CB, [128, N_STEPS], f32)
            W2 = load_const("W2", d_W2, [128, 128], f32)
            B2 = load_const("B2", d_B2, [128, 1], f32)
            W3S = load_const("W3S", d_W3S, [128, CHUNKS * 128], f32)
            B3 = load_const("B3", d_B3, [128, 1], f32)
            BB = load_const("BB", d_BB, [128, 128], f32r)
            TT = load_const("TT", d_TT, [128, 32], f32r)

            if L1_BF16:
                W1Sb = consts.tile([128, CHUNKS * 128], mybir.dt.bfloat16,
                                   tag="W1Sb", name="W1Sb")
                nc.vector.tensor_copy(W1Sb[:], W1S[:])

            if hdt == f32:
                W2m, W3Sm = W2, W3S
            else:
                W2m = consts.tile([128, 128], hdt, tag="W2m", name="W2m")
                nc.vector.tensor_copy(W2m[:], W2[:])
                W3Sm = consts.tile([128, CHUNKS * 128], hdt, tag="W3Sm", name="W3Sm")
                nc.vector.tensor_copy(W3Sm[:], W3S[:])

            # ---- persistent state ----
            y_pl = state.tile([128, FREE], f32r, tag="y_pl", name="y_pl")
            nc.gpsimd.dma_start(y_pl[:], d_YI)
            accA = state.tile([32, FREE], f32, tag="accA", name="accA")
            nc.vector.memset(accA[:], 0.0)
            accP = state.tile([128, FREE], f32, tag="accP", name="accP")
            nc.vector.memset(accP[:], 0.0)
            out3s = state.tile([128, FREE], f32r, tag="out3s", name="out3s")
            if L1_BF16:
                y_plb = state.tile([128, FREE], mybir.dt.bfloat16,
                                   tag="y_plb", name="y_plb")

            # ---- time loop ----
            # Chunk-interleaved pipeline: per chunk L1 -> (ACT evac) -> L2 ->
            # (DVE/ACT evac) -> L3-accumulate; the psum rings provide the
            # cross-chunk overlap. SBUF-only elementwise work runs on the
            # otherwise-idle GPSIMD.
            n_loop = N_STEPS if LOOP_STEPS is None else LOOP_STEPS
            for n in range(n_loop):
                dw_t = dwp.tile([128, FREE], f32, tag="dw", name="dw")
                nc.gpsimd.dma_start(dw_t[:, :], d_dWt[n, :])

                p3 = ps3.tile([128, FREE], f32, tag="p3", name="p3")
                cb_n = CB[:, n:n + 1]

                if L1_BF16:
                    nc.gpsimd.tensor_copy(y_plb[:], y_pl[:])
                    l1_w, l1_y = W1Sb, y_plb
                else:
                    l1_w, l1_y = W1S, y_pl

                def emit_L1(k):
                    p1 = ps1.tile([128, FREE], f32, tag="p1", name="p1")
                    nc.tensor.matmul(p1[:], l1_w[:, k * 128:(k + 1) * 128], l1_y[:])
                    return p1

                def emit_evac1(p1):
                    h1 = h1pool.tile([128, FREE], hdt, tag="h1", name="h1")
                    nc.scalar.activation(h1[:], p1[:], Act.Relu, bias=cb_n)
                    return h1

                def emit_L2(h1):
                    p2 = ps2.tile([128, FREE], f32, tag="p2", name="p2")
                    nc.tensor.matmul(p2[:], H(W2m[:]), H(h1[:]))
                    return p2

                def emit_evac2(k, p2):
                    h2 = h2pool.tile([128, FREE], hdt, tag="h2", name="h2")
                    if k < ACT_H2:
                        nc.scalar.activation(h2[:], p2[:], Act.Relu, bias=B2[:, 0:1])
                    else:
                        nc.vector.tensor_scalar(h2[:], p2[:], B2[:, 0:1], 0.0,
                                                Alu.add, Alu.max)
                    return h2

                def emit_L3(k, h2):
                    nc.tensor.matmul(
                        p3[:], H(W3Sm[:, k * 128:(k + 1) * 128]), H(h2[:]),
                        start=(k == 0), stop=(k == CHUNKS - 1),
                    )

                if PAIR_EMIT:
                    for j in range(CHUNKS // 2):
                        ka, kb = 2 * j, 2 * j + 1
                        p1a = emit_L1(ka)
                        h1a = emit_evac1(p1a)
                        p1b = emit_L1(kb)
                        h1b = emit_evac1(p1b)
                        p2a = emit_L2(h1a)
                        p2b = emit_L2(h1b)
                        h2a = emit_evac2(ka, p2a)
                        h2b = emit_evac2(kb, p2b)
                        emit_L3(ka, h2a)
                        emit_L3(kb, h2b)
                else:
                    for k in range(CHUNKS):
                        p1 = emit_L1(k)
                        h1 = emit_evac1(p1)
                        p2 = emit_L2(h1)
                        h2 = emit_evac2(k, p2)
                        emit_L3(k, h2)

                if SPLIT_TAIL:
                    # q rows evacuate first (ACT) and alone feed the K=32
                    # broadcast; z rows evacuate on DVE off the critical path
                    oq = tmp.tile([32, FREE], f32r, tag="oq", name="oq")
                    nc.vector.tensor_scalar(oq[:], p3[0:32, :], B3[0:32, 0:1], None, Alu.add)
                    pqpool, pqtag = (ps3, "p3") if PS2_4 else (psq, "pq")
                    pq = pqpool.tile([128, FREE], f32, tag=pqtag, name="pq")
                    nc.tensor.matmul(pq[:], BB[0:32, :], oq[:])
                    nc.vector.tensor_scalar(out3s[32:128, :], p3[32:128, :],
                                            B3[32:128, 0:1], None, Alu.add)
                else:
                    # out3s = psum3 + per-partition bias
                    nc.vector.tensor_scalar(out3s[:], p3[:], B3[:, 0:1], None, Alu.add)

                    # qrep = broadcast dt*q to component quarters (via PE)
                    pqpool, pqtag = (ps3, "p3") if PS2_4 else (psq, "pq")
                    pq = pqpool.tile([128, FREE], f32, tag=pqtag, name="pq")
                    nc.tensor.matmul(pq[:], BB[:], out3s[:])

                # ---- elementwise state update ----
                th = tmp.tile([128, FREE], f32, tag="th", name="th")
                nc.scalar.activation(th[:], y_pl[:], Act.Tanh)
                t1 = tmp.tile([128, FREE], f32, tag="t1", name="t1")
                nc.vector.tensor_scalar(t1[:], th[:], a_coef, b_coef, Alu.mult, Alu.add)
                t2 = tmp.tile([128, FREE], f32, tag="t2", name="t2")
                nc.vector.scalar_tensor_tensor(t2[:], t1[:], 1.0, dw_t[:], Alu.mult, Alu.mult)
                t3 = tmp.tile([128, FREE], f32, tag="t3", name="t3")
                nc.vector.scalar_tensor_tensor(t3[:], y_pl[:], 1.0 - DT, t2[:], Alu.mult, Alu.add)
                nc.vector.scalar_tensor_tensor(y_pl[:], t3[:], 0.0, pq[:], Alu.add, Alu.add)

                sqA = tmp.tile([32, FREE], f32, tag="sqA", name="sqA")
                p6 = tmp.tile([128, FREE], f32, tag="p6", name="p6")
                if SPLIT_TAIL:
                    nc.gpsimd.tensor_tensor(sqA[:], oq[:], oq[:], Alu.mult)
                    nc.gpsimd.tensor_tensor(accA[:], accA[:], sqA[:], Alu.add)
                    nc.gpsimd.tensor_tensor(p6[32:128, :], out3s[32:128, :],
                                            dw_t[32:128, :], Alu.mult)
                    nc.gpsimd.tensor_tensor(accP[32:128, :], accP[32:128, :],
                                            p6[32:128, :], Alu.add)
                elif GP_ACC:
                    nc.gpsimd.tensor_tensor(sqA[:], out3s[0:32, :], out3s[0:32, :], Alu.mult)
                    nc.gpsimd.tensor_tensor(accA[:], accA[:], sqA[:], Alu.add)
                    nc.gpsimd.tensor_tensor(p6[:], out3s[:], dw_t[:], Alu.mult)
                    nc.gpsimd.tensor_tensor(accP[:], accP[:], p6[:], Alu.add)
                else:
                    nc.scalar.activation(sqA[:], out3s[0:32, :], Act.Square)
                    nc.vector.scalar_tensor_tensor(accA[:], accA[:], 0.0, sqA[:], Alu.add, Alu.add)
                    nc.vector.scalar_tensor_tensor(p6[:], out3s[:], 1.0, dw_t[:], Alu.mult, Alu.mult)
                    nc.vector.scalar_tensor_tensor(accP[:], accP[:], 0.0, p6[:], Alu.add, Alu.add)

            # ---- final loss assembly ----
            ysq = tmp.tile([128, FREE], f32r, tag="ysq", name="ysq")
            nc.scalar.activation(ysq[:], y_pl[:], Act.Square)
            p_term = ps1.tile([32, FREE], f32, tag="p1", name="pterm")
            nc.tensor.matmul(p_term[:], TT[:], ysq[:])
            accPr = tmp.tile([128, FREE], f32r, tag="accPr", name="accPr")
            nc.vector.tensor_copy(accPr[:], accP[:])
            p_P = ps2.tile([32, FREE], f32, tag="p2", name="pP")
            nc.tensor.matmul(p_P[:], TT[:], accPr[:])

            Pg = tmp.tile([32, FREE], f32, tag="Pg", name="Pg")
            nc.vector.tensor_scalar(Pg[:], p_P[:], 0.0, None, Alu.add)
            Tg = tmp.tile([32, FREE], f32, tag="Tg", name="Tg")
            nc.vector.tensor_scalar(Tg[:], p_term[:], 0.0, None, Alu.add)
            D1 = tmp.tile([32, FREE], f32, tag="D1", name="D1")
            nc.vector.scalar_tensor_tensor(D1[:], accA[:], -0.5 / DT, Pg[:], Alu.mult, Alu.add)
            D2 = tmp.tile([32, FREE], f32, tag="D2", name="D2")
            nc.vector.scalar_tensor_tensor(D2[:], Tg[:], -1.0, D1[:], Alu.mult, Alu.add)
            # add Y0 (runtime input, broadcast from SCAL[0])
            sc = consts.tile([4, 1], f32, tag="SCAL", name="SCAL")
            nc.gpsimd.dma_start(sc[:], d_SC)
            y0b = consts.tile([32, 1], f32, tag="y0b", name="y0b")
            nc.gpsimd.dma_start(y0b[:], bass.AP(tensor=d_SC.tensor, offset=0, ap=[[0, 32], [1, 1]]))
            D3 = tmp.tile([32, FREE], f32, tag="D3", name="D3")
            nc.vector.tensor_scalar(D3[:], D2[:], y0b[:, 0:1], None, Alu.add)

            dsq = tmp.tile([32, FREE], f32, tag="dsq", name="dsq")
            res = state.tile([32, 1], f32, tag="res", name="res")
            nc.scalar.activation(dsq[:], D3[:], Act.Square, accum_out=res[:])
            nc.sync.dma_start(d_res, res[:])

    return nc


LAST_EXEC_NS = None
LAST_TRACE_DIR = None


def kernel(**inputs) -> np.ndarray:
    global LAST_EXEC_NS, LAST_TRACE_DIR
    from concourse.bass_utils import run_bass_kernel_spmd
    _install_wait_splitter()
    _install_ldw_opt()

    host = prep_host(inputs)

    nc = build_program()

    scal = np.array([[host["Y0"]], [-0.5 / DT], [0.1 * SQRT_DT], [0.2 * SQRT_DT]], np.float32)
    shared = dict(
        W1S=host["W1S"], CB=host["CB"], W2=host["W2"], B2=host["B2"],
        W3S=host["W3S"], B3=host["B3"], BB=host["BB"], TT=host["TT"],
        YINIT=host["YINIT"], SCAL=scal,
    )
    in_maps = []
    for r in range(N_CORES):
        m = dict(shared)
        m["dWt"] = host["dWt_cores"][r]
        in_maps.append(m)

    trace = os.environ.get("BSDE_TRACE", "0") == "1"
    kw = {}
    if trace:
        kw["trace"] = True
        kw["tmpdir"] = os.environ.get("BSDE_TRACE_DIR") or None
    out = run_bass_kernel_spmd(nc, in_maps, list(range(N_CORES)), **kw)
    LAST_EXEC_NS = getattr(out, "exec_time_ns", None)
    total = np.float64(0.0)
    for r in range(N_CORES):
        total += np.sum(out.results[r]["res"].astype(np.float64))
    return np.float32(total / BATCH)


def _build_in_maps(host):
    scal = np.array([[host["Y0"]], [-0.5 / DT], [0.1 * SQRT_DT], [0.2 * SQRT_DT]], np.float32)
    shared = dict(
        W1S=host["W1S"], CB=host["CB"], W2=host["W2"], B2=host["B2"],
        W3S=host["W3S"], B3=host["B3"], BB=host["BB"], TT=host["TT"],
        YINIT=host["YINIT"], SCAL=scal,
    )
    in_maps = []
    for r in range(N_CORES):
        m = dict(shared)
        m["dWt"] = host["dWt_cores"][r]
        in_maps.append(m)
    return in_maps


def timed_run(nc, in_maps, iters=7):
    """Mirror bass2jax.run_bass_via_pjrt's multi-core path, but keep inputs
    device-resident and time steady-state executions. Returns (results_core0,
    sorted wall times in ns per call)."""
    import time
    import jax
    from jax.sharding import Mesh, PartitionSpec, NamedSharding
    from jax.experimental.shard_map import shard_map
    from concourse import bass2jax, mybir

    bass2jax.install_neuronx_cc_hook()
    n_cores = N_CORES

    in_names, out_names, out_avals, zero_outs = [], [], [], []
    for alloc in nc.m.functions[0].allocations:
        if not isinstance(alloc, mybir.MemoryLocationSet):
            continue
        name = alloc.memorylocations[0].name
        if alloc.kind == "ExternalInput":
            in_names.append(name)
        elif alloc.kind == "ExternalOutput":
            out_names.append(name)
            shape = tuple(alloc.tensor_shape)
            dtype = mybir.dt.np(alloc.dtype)
            out_avals.append(jax.core.ShapedArray(shape, dtype))
            zero_outs.append(np.zeros(shape, dtype))
    n_params = len(in_names)
    n_outs = len(out_avals)
    all_names = in_names + out_names
    donate = tuple(range(n_params, n_params + n_outs))

    def _body(*args):
        outs = bass2jax._bass_exec_p.bind(
            *list(args),
            out_avals=tuple(out_avals),
            in_names=tuple(all_names),
            out_names=tuple(out_names),
            lowering_input_output_aliases=(),
            sim_require_finite=True,
            sim_require_nnan=True,
            nc=nc,
        )
        return tuple(outs)

    devices = jax.devices()[:n_cores]
    mesh = Mesh(np.asarray(devices), ("core",))
    in_specs = (PartitionSpec("core"),) * (n_params + n_outs)
    out_specs = (PartitionSpec("core"),) * len(out_names)
    sharded = jax.jit(
        shard_map(_body, mesh=mesh, in_specs=in_specs, out_specs=out_specs, check_rep=False),
        donate_argnums=donate,
        keep_unused=True,
    )
    concat_in = [
        np.concatenate([np.asarray(in_maps[c][nm]) for c in range(n_cores)], axis=0)
        for nm in in_names
    ]
    sh = NamedSharding(mesh, PartitionSpec("core"))
    dev_in = [jax.device_put(a, sh) for a in concat_in]
    concat_zeros = [np.zeros((n_cores * z.shape[0], *z.shape[1:]), z.dtype) for z in zero_outs]

    out = sharded(*dev_in, *concat_zeros)   # warm-up / compile
    jax.block_until_ready(out)
    times = []
    for _ in range(iters):
        zz = [np.zeros((n_cores * z.shape[0], *z.shape[1:]), z.dtype) for z in zero_outs]
        t0 = time.perf_counter_ns()
        out = sharded(*dev_in, *zz)
        jax.block_until_ready(out)
        times.append(time.perf_counter_ns() - t0)
    res0 = {
        nm: np.asarray(out[i]).reshape(n_cores, *out_avals[i].shape)
        for i, nm in enumerate(out_names)
    }
    return res0, sorted(times)


if __name__ == "__main__":
    rng = np.random.default_rng(0)
    fake = {
        "y0": rng.standard_normal(3).astype(np.float32),
        "Y0": np.zeros((1, 1), np.float32),
        "qW1": rng.standard_normal((4, 64)).astype(np.float32) * 0.5,
        "qb1": np.zeros(64, np.float32),
        "qW2": rng.standard_normal((64, 64)).astype(np.float32) * 0.12,
        "qb2": np.zeros(64, np.float32),
        "qW3": rng.standard_normal((64, 1)).astype(np.float32) * 0.12,
        "qb3": np.zeros(1, np.float32),
        "zW1": rng.standard_normal((4, 64)).astype(np.float32) * 0.5,
        "zb1": np.zeros(64, np.float32),
        "zW2": rng.standard_normal((64, 64)).astype(np.float32) * 0.12,
        "zb2": np.zeros(64, np.float32),
        "zW3": rng.standard_normal((64, 3)).astype(np.float32) * 0.12,
        "zb3": np.zeros(3, np.float32),
        "dW": rng.standard_normal((N_STEPS, BATCH, 3)).astype(np.float32),
    }
    print(kernel(**fake))



# revision 30
# speedup vs baseline: 1.1989x; 1.1989x over previous
"""DeepBSDE forward-loss kernel for Trainium2 (8 NeuronCores, data-parallel).

Math (per sample b, 50 steps, dt=0.02):
    x_n = [t_n, y_n]                       (4 features)
    z_n = MLP_z(x_n)   (4->64->64->3, relu)
    q_n = MLP_q(x_n)   (4->64->64->1, relu)
    y_{n+1} = (1-dt) y_n + dt q_n + (0.2 + 0.1 tanh(y_n)) * sqrt(dt) * dW_n
    Y_final = Y0 - 0.5 dt sum_n q_n^2 + sum_n z_n . (sqrt(dt) dW_n)
    out = mean_b (Y_final - |y_final|^2)^2

Device layout (per core, B_loc = 16384 = 32 chunks x 512):
    every per-sample state lives in a [128, 512] SBUF tile:
      partition k        (k in 0..31)   : q-slot of chunk k
      partition 32+32i+k (i in 0..2)    : vector component i of chunk k
      free c                            : sample index b = k*512 + c
    The two MLPs are fused: hidden = [q-hidden(64) ; z-hidden(64)] = 128.
    Per step the PE streams: L1 (K=3), L2 (K=128), L3 (per-chunk sparse
    [128,128] stationaries accumulated into ONE packed psum bank), plus one
    broadcast matmul that replicates dt*q into the 3 component quarters.
"""

import sys
import os

for _p in ("/opt/trn_rl_repo", "/root/.axon_site/_ro/trn_rl_repo"):
    if os.path.isdir(_p) and _p not in sys.path:
        sys.path.insert(0, _p)

import numpy as np

DT = 0.02
SQRT_DT = float(np.sqrt(np.float32(DT)))
N_STEPS = 50
BATCH = 131072
DIM = 3
N_CORES = 8
B_LOC = BATCH // N_CORES          # 16384
CHUNKS = 32
FREE = B_LOC // CHUNKS            # 512

# dtype knobs.
#  - L1/BB/TT matmuls read fp32 state; run them as float32r (same 4-byte
#    storage, 1 cycle/row on the PE at moving size >= 256 vs 4 for fp32).
#  - hidden activations h1/h2 and the L2/L3 weights run in bf16: same PE
#    rate as f32r but half the ACT/DVE evacuation cost and half the
#    weight-load traffic.
MM_HID_F32 = os.environ.get("BSDE_HID_F32", "0") == "1"

# how many h2 evacuations run on the scalar (ACT) engine instead of DVE,
# to balance the two engines' per-step load.
ACT_H2 = int(os.environ.get("BSDE_ACT_H2", "2"))

# offload the final-reduction accumulation ops (sqA product, accA/accP adds,
# p6 product) to the otherwise-idle GPSIMD engine.
GP_ACC = os.environ.get("BSDE_GP_ACC", "1") == "1"

# L1 matmuls in bf16 (stationary + a per-step bf16 copy of y): halves the
# f32r weight-load time on the PE at a tiny precision cost.
L1_BF16 = os.environ.get("BSDE_L1_BF16", "0") == "1"

# split the out3s evacuation: q rows via ACT feed a K=32 broadcast matmul so
# the z-row evacuation leaves the step-boundary critical path.
SPLIT_TAIL = os.environ.get("BSDE_SPLIT_TAIL", "0") == "1"

# emit PE work pair-adjacent (L1,L1,L2,L2,L3,L3 per chunk pair) so the two
# W2 loads sit back-to-back for weight-load pull-ahead.
PAIR_EMIT = os.environ.get("BSDE_PAIR_EMIT", "0") == "1"

# deeper activation rings to absorb evacuation jitter (SBUF has headroom)
RING_H = os.environ.get("BSDE_RING_H", "1") == "1"

# give ps2 a 4th bank by allocating pq from ps3's ring (p3's bank is free by
# the time the broadcast runs; the WAR dep is exactly the required ordering)
PS2_4 = os.environ.get("BSDE_PS2_4", "1") == "1"

# evacuate out3s on the ACT engine (Identity + vector bias): at the step tail
# ACT's queue is empty while DVE is still draining h2 evacuations, so the
# p3 -> out3s -> broadcast -> y-add chain starts sooner.
TAIL_ACT = os.environ.get("BSDE_TAIL_ACT", "1") == "1"

# emit the diffusion chain (tanh, t1, t2, t3) spread through the chunk loop
# so only the final y-add remains in the step tail's DVE queue.
T3_EARLY = os.environ.get("BSDE_T3_EARLY", "0") == "1"

# If set (by the timing harness), the device loop runs this many steps while
# all I/O shapes stay identical — lets wall-clock differencing isolate the
# per-step device time from RPC/transfer overhead.
LOOP_STEPS = None


def _np_f32(x):
    return np.ascontiguousarray(np.asarray(x, dtype=np.float32))


def prep_host(inputs):
    """Build all device-side arrays (numpy fp32) from the raw problem inputs."""
    i = {k: _np_f32(v) for k, v in inputs.items()}
    qW1, qb1 = i["qW1"], i["qb1"]
    qW2, qb2 = i["qW2"], i["qb2"]
    qW3, qb3 = i["qW3"], i["qb3"]
    zW1, zb1 = i["zW1"], i["zb1"]
    zW2, zb2 = i["zW2"], i["zb2"]
    zW3, zb3 = i["zW3"], i["zb3"]
    y0 = i["y0"]
    Y0 = float(i["Y0"].reshape(-1)[0])
    dW = i["dW"]

    W1cat = np.concatenate([qW1, zW1], axis=1)          # [4, 128]
    b1cat = np.concatenate([qb1, zb1])                  # [128]

    # L1 reads the packed y state directly: per-chunk sparse stationaries.
    # chunk k's component i lives at partition 32 + 32*i + k.
    W1S = np.zeros((CHUNKS, 128, 128), np.float32)
    for k in range(CHUNKS):
        for comp in range(3):
            W1S[k, 32 + 32 * comp + k, :] = W1cat[1 + comp, :]
    W1S = np.ascontiguousarray(W1S.transpose(1, 0, 2).reshape(128, CHUNKS * 128))

    # per-step bias for the L1 relu evacuation: c_n = t_n * W1cat[0] + b1cat
    ts = (np.arange(N_STEPS, dtype=np.float32) * np.float32(DT))
    CB = (ts[None, :] * W1cat[0][:, None] + b1cat[:, None]).astype(np.float32)  # [128, 50]

    W2 = np.zeros((128, 128), np.float32)
    W2[0:64, 0:64] = qW2
    W2[64:128, 64:128] = zW2
    B2 = b1cat * 0.0
    B2 = np.concatenate([qb2, zb2]).astype(np.float32).reshape(128, 1)

    # L3: per-chunk sparse stationaries [128, 32*128]
    W3S = np.zeros((CHUNKS, 128, 128), np.float32)
    for k in range(CHUNKS):
        W3S[k, 0:64, k] = DT * qW3[:, 0]
        for comp in range(3):
            W3S[k, 64:128, 32 + 32 * comp + k] = SQRT_DT * zW3[:, comp]
    W3S = np.ascontiguousarray(W3S.transpose(1, 0, 2).reshape(128, CHUNKS * 128))

    B3 = np.zeros((128, 1), np.float32)
    B3[0:32, 0] = DT * qb3[0]
    for comp in range(3):
        B3[32 + 32 * comp:64 + 32 * comp, 0] = SQRT_DT * zb3[comp]

    # broadcast matmul: qrep[32+32i+k] = out3s[k]
    BB = np.zeros((128, 128), np.float32)
    for k in range(CHUNKS):
        for comp in range(3):
            BB[k, 32 + 32 * comp + k] = 1.0

    # final reduction: col j sums the 3 components of chunk j
    TT = np.zeros((128, 32), np.float32)
    for j in range(CHUNKS):
        for comp in range(3):
            TT[32 + 32 * comp + j, j] = 1.0

    # initial y state, broadcast to full [128, 512] (q-slot rows zero)
    YINIT = np.zeros((128, FREE), np.float32)
    for comp in range(3):
        YINIT[32 + 32 * comp:64 + 32 * comp, :] = y0[comp]

    # per-core dW, transposed to [steps, comp, b_loc]; row block 0 (the q-slot
    # partitions) is zeros so a single full-tile DMA initializes everything
    dWt_cores = []
    for r in range(N_CORES):
        sl = dW[:N_STEPS, r * B_LOC:(r + 1) * B_LOC, :]     # [N_STEPS, B_loc, 3]
        t = np.zeros((N_STEPS, 4 * B_LOC), np.float32)
        t[:, B_LOC:] = sl.transpose(0, 2, 1).reshape(N_STEPS, 3 * B_LOC)
        dWt_cores.append(t)

    return dict(
        W1S=W1S, CB=CB, W2=W2, B2=B2, W3S=W3S, B3=B3, BB=BB, TT=TT,
        YINIT=YINIT, dWt_cores=dWt_cores, Y0=Y0,
    )


def _split_sync_waits(bir: dict) -> dict:
    """Walrus in this toolchain accepts only ~1 sync wait per instruction.
    Hoist extra waits onto standalone EventSemaphore instructions inserted
    just before, on the same engine (waits-only, so semantics unchanged)."""
    n = 0
    for fn in bir.get("functions", []):
        for bb in fn.get("blocks", []):
            out = []
            for ins in bb.get("instructions", []):
                si = ins.get("sync_info")
                waits = (si or {}).get("on_wait") or []
                if len(waits) > 1:
                    for w in waits[:-1]:
                        n += 1
                        out.append({
                            "engine": ins["engine"],
                            "ins": [],
                            "outs": [],
                            "name": f"bsdewait{n}_{ins['name']}",
                            "opcode": "EventSemaphore",
                            "debug": ins.get("debug", 0),
                            "sync_info": {"on_update": [], "on_wait": [w]},
                        })
                    si["on_wait"] = [waits[-1]]
                out.append(ins)
            bb["instructions"] = out
    return bir


def _install_ldw_opt():
    """walrus is invoked with --enable-ldw-opt=false; flip it on so repeated/
    adjacent stationary loads are optimized (gated by BSDE_LDW_OPT)."""
    from concourse import bass_utils
    if os.environ.get("BSDE_LDW_OPT", "0") != "1":
        return
    if getattr(bass_utils, "_bsde_ldwopt_installed", False):
        return
    orig = bass_utils.run_command

    def wrapped(cmd, **kw):
        if isinstance(cmd, list):
            cmd = ["--enable-ldw-opt=true" if c == "--enable-ldw-opt=false" else c
                   for c in cmd]
        return orig(cmd, **kw)

    bass_utils.run_command = wrapped
    bass_utils._bsde_ldwopt_installed = True


def _install_wait_splitter():
    import json as _json
    from concourse import bass2jax, bass_utils
    if getattr(bass_utils, "_bsde_split_installed", False):
        return
    orig = bass_utils.compile_bir_kernel

    def wrapped(bir_json, tmpdir, neff_name="file.neff"):
        bir = _json.loads(bir_json)
        _split_sync_waits(bir)
        return orig(_json.dumps(bir).encode(), tmpdir, neff_name)

    bass_utils.compile_bir_kernel = wrapped
    bass2jax.compile_bir_kernel = wrapped
    bass_utils._bsde_split_installed = True


def build_program():
    """Build the Bass program (same for all cores). Returns (nc, meta)."""
    from concourse import bass, mybir, tile

    f32 = mybir.dt.float32
    f32r = mybir.dt.float32r
    hdt = f32 if MM_HID_F32 else mybir.dt.bfloat16
    Alu = mybir.AluOpType
    Act = mybir.ActivationFunctionType

    def R(ap):
        # reinterpret fp32 data as float32r for full-rate PE streaming
        return ap.bitcast(f32r)

    def H(ap):
        # hidden-path operand: native bf16, or f32->f32r bitcast in fallback
        return R(ap) if hdt == f32 else ap

    nc = bass.Bass("TRN2", target_bir_lowering=False, debug=False)

    # --- dram I/O ---
    d_dWt = nc.dram_tensor("dWt", [N_STEPS, 4 * B_LOC], f32, kind="ExternalInput").ap()
    d_W1S = nc.dram_tensor("W1S", [128, CHUNKS * 128], f32r, kind="ExternalInput").ap()
    d_CB = nc.dram_tensor("CB", [128, N_STEPS], f32, kind="ExternalInput").ap()
    d_W2 = nc.dram_tensor("W2", [128, 128], f32, kind="ExternalInput").ap()
    d_B2 = nc.dram_tensor("B2", [128, 1], f32, kind="ExternalInput").ap()
    d_W3S = nc.dram_tensor("W3S", [128, CHUNKS * 128], f32, kind="ExternalInput").ap()
    d_B3 = nc.dram_tensor("B3", [128, 1], f32, kind="ExternalInput").ap()
    d_BB = nc.dram_tensor("BB", [128, 128], f32r, kind="ExternalInput").ap()
    d_TT = nc.dram_tensor("TT", [128, 32], f32r, kind="ExternalInput").ap()
    d_YI = nc.dram_tensor("YINIT", [128, FREE], f32r, kind="ExternalInput").ap()
    d_SC = nc.dram_tensor("SCAL", [4, 1], f32, kind="ExternalInput").ap()  # [Y0; -0.5/dt; a; b]
    d_res = nc.dram_tensor("res", [32, 1], f32, kind="ExternalOutput").ap()

    a_coef = 0.1 * SQRT_DT
    b_coef = 0.2 * SQRT_DT

    with tile.TileContext(nc) as tc:
        with (
            tc.tile_pool(name="consts", bufs=1) as consts,
            tc.tile_pool(name="state", bufs=1) as state,
            tc.tile_pool(name="h1p", bufs=(6 if RING_H else 3)) as h1pool,
            tc.tile_pool(name="h2p", bufs=(8 if RING_H else 4)) as h2pool,
            tc.tile_pool(name="tmp", bufs=1) as tmp,
            tc.tile_pool(name="dwp", bufs=8) as dwp,
            tc.tile_pool(name="ps1", bufs=3, space="PSUM") as ps1,
            tc.tile_pool(name="ps2", bufs=(4 if PS2_4 else 3), space="PSUM") as ps2,
            tc.tile_pool(name="ps3", bufs=1, space="PSUM") as ps3,
            tc.tile_pool(name="psq", bufs=1, space="PSUM") as psq,
        ):
            # ---- load constants into SBUF ----
            _dma_eng = [nc.sync, nc.scalar, nc.gpsimd]
            _dma_i = [0]

            def load_const(name, dram_ap, shape, dt_):
                t = consts.tile(shape, dt_, tag=name)
                eng = _dma_eng[_dma_i[0] % len(_dma_eng)]
                _dma_i[0] += 1
                eng.dma_start(t[:], dram_ap)
                return t

            W1S = load_const("W1S", d_W1S, [128, CHUNKS * 128], f32r)
            CB = load_const("CB", d_CB, [128, N_STEPS], f32)
            W2 = load_const("W2", d_W2, [128, 128], f32)
            B2 = load_const("B2", d_B2, [128, 1], f32)
            W3S = load_const("W3S", d_W3S, [128, CHUNKS * 128], f32)
            B3 = load_const("B3", d_B3, [128, 1], f32)
            BB = load_const("BB", d_BB, [128, 128], f32r)
            TT = load_const("TT", d_TT, [128, 32], f32r)

            if L1_BF16:
                W1Sb = consts.tile([128, CHUNKS * 128], mybir.dt.bfloat16,
                                   tag="W1Sb", name="W1Sb")
                nc.vector.tensor_copy(W1Sb[:], W1S[:])

            if hdt == f32:
                W2m, W3Sm = W2, W3S
            else:
                W2m = consts.tile([128, 128], hdt, tag="W2m", name="W2m")
                nc.vector.tensor_copy(W2m[:], W2[:])
                W3Sm = consts.tile([128, CHUNKS * 128], hdt, tag="W3Sm", name="W3Sm")
                nc.vector.tensor_copy(W3Sm[:], W3S[:])

            # ---- persistent state ----
            y_pl = state.tile([128, FREE], f32r, tag="y_pl", name="y_pl")
            nc.gpsimd.dma_start(y_pl[:], d_YI)
            accA = state.tile([32, FREE], f32, tag="accA", name="accA")
            nc.vector.memset(accA[:], 0.0)
            accP = state.tile([128, FREE], f32, tag="accP", name="accP")
            nc.vector.memset(accP[:], 0.0)
            out3s = state.tile([128, FREE], f32r, tag="out3s", name="out3s")
            if L1_BF16:
                y_plb = state.tile([128, FREE], mybir.dt.bfloat16,
                                   tag="y_plb", name="y_plb")

            # ---- time loop ----
            # Chunk-interleaved pipeline: per chunk L1 -> (ACT evac) -> L2 ->
            # (DVE/ACT evac) -> L3-accumulate; the psum rings provide the
            # cross-chunk overlap. SBUF-only elementwise work runs on the
            # otherwise-idle GPSIMD.
            n_loop = N_STEPS if LOOP_STEPS is None else LOOP_STEPS
            for n in range(n_loop):
                dw_t = dwp.tile([128, FREE], f32, tag="dw", name="dw")
                nc.gpsimd.dma_start(dw_t[:, :], d_dWt[n, :])

                p3 = ps3.tile([128, FREE], f32, tag="p3", name="p3")
                cb_n = CB[:, n:n + 1]

                if L1_BF16:
                    nc.gpsimd.tensor_copy(y_plb[:], y_pl[:])
                    l1_w, l1_y = W1Sb, y_plb
                else:
                    l1_w, l1_y = W1S, y_pl

                def emit_L1(k):
                    p1 = ps1.tile([128, FREE], f32, tag="p1", name="p1")
                    nc.tensor.matmul(p1[:], l1_w[:, k * 128:(k + 1) * 128], l1_y[:])
                    return p1

                def emit_evac1(p1):
                    h1 = h1pool.tile([128, FREE], hdt, tag="h1", name="h1")
                    nc.scalar.activation(h1[:], p1[:], Act.Relu, bias=cb_n)
                    return h1

                def emit_L2(h1):
                    p2 = ps2.tile([128, FREE], f32, tag="p2", name="p2")
                    nc.tensor.matmul(p2[:], H(W2m[:]), H(h1[:]))
                    return p2

                def emit_evac2(k, p2):
                    h2 = h2pool.tile([128, FREE], hdt, tag="h2", name="h2")
                    if k < ACT_H2:
                        nc.scalar.activation(h2[:], p2[:], Act.Relu, bias=B2[:, 0:1])
                    else:
                        nc.vector.tensor_scalar(h2[:], p2[:], B2[:, 0:1], 0.0,
                                                Alu.add, Alu.max)
                    return h2

                def emit_L3(k, h2):
                    nc.tensor.matmul(
                        p3[:], H(W3Sm[:, k * 128:(k + 1) * 128]), H(h2[:]),
                        start=(k == 0), stop=(k == CHUNKS - 1),
                    )

                if PAIR_EMIT:
                    for j in range(CHUNKS // 2):
                        ka, kb = 2 * j, 2 * j + 1
                        p1a = emit_L1(ka)
                        h1a = emit_evac1(p1a)
                        p1b = emit_L1(kb)
                        h1b = emit_evac1(p1b)
                        p2a = emit_L2(h1a)
                        p2b = emit_L2(h1b)
                        h2a = emit_evac2(ka, p2a)
                        h2b = emit_evac2(kb, p2b)
                        emit_L3(ka, h2a)
                        emit_L3(kb, h2b)
                else:
                    for k in range(CHUNKS):
                        if T3_EARLY:
                            if k == 10:
                                th = tmp.tile([128, FREE], f32, tag="th", name="th")
                                nc.scalar.activation(th[:], y_pl[:], Act.Tanh)
                            elif k == 14:
                                t1 = tmp.tile([128, FREE], f32, tag="t1", name="t1")
                                nc.vector.tensor_scalar(t1[:], th[:], a_coef, b_coef,
                                                        Alu.mult, Alu.add)
                            elif k == 18:
                                t2 = tmp.tile([128, FREE], f32, tag="t2", name="t2")
                                nc.gpsimd.tensor_tensor(t2[:], t1[:], dw_t[:], Alu.mult)
                            elif k == 24:
                                t3 = tmp.tile([128, FREE], f32, tag="t3", name="t3")
                                nc.vector.scalar_tensor_tensor(t3[:], y_pl[:], 1.0 - DT,
                                                               t2[:], Alu.mult, Alu.add)
                        p1 = emit_L1(k)
                        h1 = emit_evac1(p1)
                        p2 = emit_L2(h1)
                        h2 = emit_evac2(k, p2)
                        emit_L3(k, h2)

                if SPLIT_TAIL:
                    # q rows evacuate first (ACT) and alone feed the K=32
                    # broadcast; z rows evacuate on DVE off the critical path
                    oq = tmp.tile([32, FREE], f32r, tag="oq", name="oq")
                    nc.vector.tensor_scalar(oq[:], p3[0:32, :], B3[0:32, 0:1], None, Alu.add)
                    pqpool, pqtag = (ps3, "p3") if PS2_4 else (psq, "pq")
                    pq = pqpool.tile([128, FREE], f32, tag=pqtag, name="pq")
                    nc.tensor.matmul(pq[:], BB[0:32, :], oq[:])
                    nc.vector.tensor_scalar(out3s[32:128, :], p3[32:128, :],
                                            B3[32:128, 0:1], None, Alu.add)
                else:
                    # out3s = psum3 + per-partition bias
                    if TAIL_ACT:
                        nc.scalar.activation(out3s[:], p3[:], Act.Identity, bias=B3[:, 0:1])
                    else:
                        nc.vector.tensor_scalar(out3s[:], p3[:], B3[:, 0:1], None, Alu.add)

                    # qrep = broadcast dt*q to component quarters (via PE)
                    pqpool, pqtag = (ps3, "p3") if PS2_4 else (psq, "pq")
                    pq = pqpool.tile([128, FREE], f32, tag=pqtag, name="pq")
                    nc.tensor.matmul(pq[:], BB[:], out3s[:])

                # ---- elementwise state update ----
                if not T3_EARLY:
                    th = tmp.tile([128, FREE], f32, tag="th", name="th")
                    nc.scalar.activation(th[:], y_pl[:], Act.Tanh)
                    t1 = tmp.tile([128, FREE], f32, tag="t1", name="t1")
                    nc.vector.tensor_scalar(t1[:], th[:], a_coef, b_coef, Alu.mult, Alu.add)
                    t2 = tmp.tile([128, FREE], f32, tag="t2", name="t2")
                    nc.vector.scalar_tensor_tensor(t2[:], t1[:], 1.0, dw_t[:], Alu.mult, Alu.mult)
                    t3 = tmp.tile([128, FREE], f32, tag="t3", name="t3")
                    nc.vector.scalar_tensor_tensor(t3[:], y_pl[:], 1.0 - DT, t2[:], Alu.mult, Alu.add)
                nc.vector.scalar_tensor_tensor(y_pl[:], t3[:], 0.0, pq[:], Alu.add, Alu.add)

                sqA = tmp.tile([32, FREE], f32, tag="sqA", name="sqA")
                p6 = tmp.tile([128, FREE], f32, tag="p6", name="p6")
                if SPLIT_TAIL:
                    nc.gpsimd.tensor_tensor(sqA[:], oq[:], oq[:], Alu.mult)
                    nc.gpsimd.tensor_tensor(accA[:], accA[:], sqA[:], Alu.add)
                    nc.gpsimd.tensor_tensor(p6[32:128, :], out3s[32:128, :],
                                            dw_t[32:128, :], Alu.mult)
                    nc.gpsimd.tensor_tensor(accP[32:128, :], accP[32:128, :],
                                            p6[32:128, :], Alu.add)
                elif GP_ACC:
                    nc.gpsimd.tensor_tensor(sqA[:], out3s[0:32, :], out3s[0:32, :], Alu.mult)
                    nc.gpsimd.tensor_tensor(accA[:], accA[:], sqA[:], Alu.add)
                    nc.gpsimd.tensor_tensor(p6[:], out3s[:], dw_t[:], Alu.mult)
                    nc.gpsimd.tensor_tensor(accP[:], accP[:], p6[:], Alu.add)
                else:
                    nc.scalar.activation(sqA[:], out3s[0:32, :], Act.Square)
                    nc.vector.scalar_tensor_tensor(accA[:], accA[:], 0.0, sqA[:], Alu.add, Alu.add)
                    nc.vector.scalar_tensor_tensor(p6[:], out3s[:], 1.0, dw_t[:], Alu.mult, Alu.mult)
                    nc.vector.scalar_tensor_tensor(accP[:], accP[:], 0.0, p6[:], Alu.add, Alu.add)

            # ---- final loss assembly ----
            ysq = tmp.tile([128, FREE], f32r, tag="ysq", name="ysq")
            nc.scalar.activation(ysq[:], y_pl[:], Act.Square)
            p_term = ps1.tile([32, FREE], f32, tag="p1", name="pterm")
            nc.tensor.matmul(p_term[:], TT[:], ysq[:])
            accPr = tmp.tile([128, FREE], f32r, tag="accPr", name="accPr")
            nc.vector.tensor_copy(accPr[:], accP[:])
            p_P = ps2.tile([32, FREE], f32, tag="p2", name="pP")
            nc.tensor.matmul(p_P[:], TT[:], accPr[:])

            Pg = tmp.tile([32, FREE], f32, tag="Pg", name="Pg")
            nc.vector.tensor_scalar(Pg[:], p_P[:], 0.0, None, Alu.add)
            Tg = tmp.tile([32, FREE], f32, tag="Tg", name="Tg")
            nc.vector.tensor_scalar(Tg[:], p_term[:], 0.0, None, Alu.add)
            D1 = tmp.tile([32, FREE], f32, tag="D1", name="D1")
            nc.vector.scalar_tensor_tensor(D1[:], accA[:], -0.5 / DT, Pg[:], Alu.mult, Alu.add)
            D2 = tmp.tile([32, FREE], f32, tag="D2", name="D2")
            nc.vector.scalar_tensor_tensor(D2[:], Tg[:], -1.0, D1[:], Alu.mult, Alu.add)
            # add Y0 (runtime input, broadcast from SCAL[0])
            sc = consts.tile([4, 1], f32, tag="SCAL", name="SCAL")
            nc.gpsimd.dma_start(sc[:], d_SC)
            y0b = consts.tile([32, 1], f32, tag="y0b", name="y0b")
            nc.gpsimd.dma_start(y0b[:], bass.AP(tensor=d_SC.tensor, offset=0, ap=[[0, 32], [1, 1]]))
            D3 = tmp.tile([32, FREE], f32, tag="D3", name="D3")
            nc.vector.tensor_scalar(D3[:], D2[:], y0b[:, 0:1], None, Alu.add)

            dsq = tmp.tile([32, FREE], f32, tag="dsq", name="dsq")
            res = state.tile([32, 1], f32, tag="res", name="res")
            nc.scalar.activation(dsq[:], D3[:], Act.Square, accum_out=res[:])
            nc.sync.dma_start(d_res, res[:])

    return nc


LAST_EXEC_NS = None
LAST_TRACE_DIR = None


def kernel(**inputs) -> np.ndarray:
    global LAST_EXEC_NS, LAST_TRACE_DIR
    from concourse.bass_utils import run_bass_kernel_spmd
    _install_wait_splitter()
    _install_ldw_opt()

    host = prep_host(inputs)

    nc = build_program()

    scal = np.array([[host["Y0"]], [-0.5 / DT], [0.1 * SQRT_DT], [0.2 * SQRT_DT]], np.float32)
    shared = dict(
        W1S=host["W1S"], CB=host["CB"], W2=host["W2"], B2=host["B2"],
        W3S=host["W3S"], B3=host["B3"], BB=host["BB"], TT=host["TT"],
        YINIT=host["YINIT"], SCAL=scal,
    )
    in_maps = []
    for r in range(N_CORES):
        m = dict(shared)
        m["dWt"] = host["dWt_cores"][r]
        in_maps.append(m)

    trace = os.environ.get("BSDE_TRACE", "0") == "1"
    kw = {}
    if trace:
        kw["trace"] = True
        kw["tmpdir"] = os.environ.get("BSDE_TRACE_DIR") or None
    out = run_bass_kernel_spmd(nc, in_maps, list(range(N_CORES)), **kw)
    LAST_EXEC_NS = getattr(out, "exec_time_ns", None)
    total = np.float64(0.0)
    for r in range(N_CORES):
        total += np.sum(out.results[r]["res"].astype(np.float64))
    return np.float32(total / BATCH)


def _build_in_maps(host):
    scal = np.array([[host["Y0"]], [-0.5 / DT], [0.1 * SQRT_DT], [0.2 * SQRT_DT]], np.float32)
    shared = dict(
        W1S=host["W1S"], CB=host["CB"], W2=host["W2"], B2=host["B2"],
        W3S=host["W3S"], B3=host["B3"], BB=host["BB"], TT=host["TT"],
        YINIT=host["YINIT"], SCAL=scal,
    )
    in_maps = []
    for r in range(N_CORES):
        m = dict(shared)
        m["dWt"] = host["dWt_cores"][r]
        in_maps.append(m)
    return in_maps


def timed_run(nc, in_maps, iters=7):
    """Mirror bass2jax.run_bass_via_pjrt's multi-core path, but keep inputs
    device-resident and time steady-state executions. Returns (results_core0,
    sorted wall times in ns per call)."""
    import time
    import jax
    from jax.sharding import Mesh, PartitionSpec, NamedSharding
    from jax.experimental.shard_map import shard_map
    from concourse import bass2jax, mybir

    bass2jax.install_neuronx_cc_hook()
    n_cores = N_CORES

    in_names, out_names, out_avals, zero_outs = [], [], [], []
    for alloc in nc.m.functions[0].allocations:
        if not isinstance(alloc, mybir.MemoryLocationSet):
            continue
        name = alloc.memorylocations[0].name
        if alloc.kind == "ExternalInput":
            in_names.append(name)
        elif alloc.kind == "ExternalOutput":
            out_names.append(name)
            shape = tuple(alloc.tensor_shape)
            dtype = mybir.dt.np(alloc.dtype)
            out_avals.append(jax.core.ShapedArray(shape, dtype))
            zero_outs.append(np.zeros(shape, dtype))
    n_params = len(in_names)
    n_outs = len(out_avals)
    all_names = in_names + out_names
    donate = tuple(range(n_params, n_params + n_outs))

    def _body(*args):
        outs = bass2jax._bass_exec_p.bind(
            *list(args),
            out_avals=tuple(out_avals),
            in_names=tuple(all_names),
            out_names=tuple(out_names),
            lowering_input_output_aliases=(),
            sim_require_finite=True,
            sim_require_nnan=True,
            nc=nc,
        )
        return tuple(outs)

    devices = jax.devices()[:n_cores]
    mesh = Mesh(np.asarray(devices), ("core",))
    in_specs = (PartitionSpec("core"),) * (n_params + n_outs)
    out_specs = (PartitionSpec("core"),) * len(out_names)
    sharded = jax.jit(
        shard_map(_body, mesh=mesh, in_specs=in_specs, out_specs=out_specs, check_rep=False),
        donate_argnums=donate,
        keep_unused=True,
    )
    concat_in = [
        np.concatenate([np.asarray(in_maps[c][nm]) for c in range(n_cores)], axis=0)
        for nm in in_names
    ]
    sh = NamedSharding(mesh, PartitionSpec("core"))
    dev_in = [jax.device_put(a, sh) for a in concat_in]
    concat_zeros = [np.zeros((n_cores * z.shape[0], *z.shape[1:]), z.dtype) for z in zero_outs]

    out = sharded(*dev_in, *concat_zeros)   # warm-up / compile
    jax.block_until_ready(out)
    times = []
    for _ in range(iters):
        zz = [np.zeros((n_cores * z.shape[0], *z.shape[1:]), z.dtype) for z in zero_outs]
        t0 = time.perf_counter_ns()
        out = sharded(*dev_in, *zz)
        jax.block_until_ready(out)
        times.append(time.perf_counter_ns() - t0)
    res0 = {
        nm: np.asarray(out[i]).reshape(n_cores, *out_avals[i].shape)
        for i, nm in enumerate(out_names)
    }
    return res0, sorted(times)


if __name__ == "__main__":
    rng = np.random.default_rng(0)
    fake = {
        "y0": rng.standard_normal(3).astype(np.float32),
        "Y0": np.zeros((1, 1), np.float32),
        "qW1": rng.standard_normal((4, 64)).astype(np.float32) * 0.5,
        "qb1": np.zeros(64, np.float32),
        "qW2": rng.standard_normal((64, 64)).astype(np.float32) * 0.12,
        "qb2": np.zeros(64, np.float32),
        "qW3": rng.standard_normal((64, 1)).astype(np.float32) * 0.12,
        "qb3": np.zeros(1, np.float32),
        "zW1": rng.standard_normal((4, 64)).astype(np.float32) * 0.5,
        "zb1": np.zeros(64, np.float32),
        "zW2": rng.standard_normal((64, 64)).astype(np.float32) * 0.12,
        "zb2": np.zeros(64, np.float32),
        "zW3": rng.standard_normal((64, 3)).astype(np.float32) * 0.12,
        "zb3": np.zeros(3, np.float32),
        "dW": rng.standard_normal((N_STEPS, BATCH, 3)).astype(np.float32),
    }
    print(kernel(**fake))



# revision 33
# speedup vs baseline: 1.2105x; 1.0097x over previous
"""DeepBSDE forward-loss kernel for Trainium2 (8 NeuronCores, data-parallel).

Math (per sample b, 50 steps, dt=0.02):
    x_n = [t_n, y_n]                       (4 features)
    z_n = MLP_z(x_n)   (4->64->64->3, relu)
    q_n = MLP_q(x_n)   (4->64->64->1, relu)
    y_{n+1} = (1-dt) y_n + dt q_n + (0.2 + 0.1 tanh(y_n)) * sqrt(dt) * dW_n
    Y_final = Y0 - 0.5 dt sum_n q_n^2 + sum_n z_n . (sqrt(dt) dW_n)
    out = mean_b (Y_final - |y_final|^2)^2

Device layout (per core, B_loc = 16384 = 32 chunks x 512):
    every per-sample state lives in a [128, 512] SBUF tile:
      partition k        (k in 0..31)   : q-slot of chunk k
      partition 32+32i+k (i in 0..2)    : vector component i of chunk k
      free c                            : sample index b = k*512 + c
    The two MLPs are fused: hidden = [q-hidden(64) ; z-hidden(64)] = 128.
    Per step the PE streams: L1 (K=3), L2 (K=128), L3 (per-chunk sparse
    [128,128] stationaries accumulated into ONE packed psum bank), plus one
    broadcast matmul that replicates dt*q into the 3 component quarters.
"""

import sys
import os

for _p in ("/opt/trn_rl_repo", "/root/.axon_site/_ro/trn_rl_repo"):
    if os.path.isdir(_p) and _p not in sys.path:
        sys.path.insert(0, _p)

import numpy as np

DT = 0.02
SQRT_DT = float(np.sqrt(np.float32(DT)))
N_STEPS = 50
BATCH = 131072
DIM = 3
N_CORES = 8
B_LOC = BATCH // N_CORES          # 16384
CHUNKS = 32
FREE = B_LOC // CHUNKS            # 512

# dtype knobs.
#  - L1/BB/TT matmuls read fp32 state; run them as float32r (same 4-byte
#    storage, 1 cycle/row on the PE at moving size >= 256 vs 4 for fp32).
#  - hidden activations h1/h2 and the L2/L3 weights run in bf16: same PE
#    rate as f32r but half the ACT/DVE evacuation cost and half the
#    weight-load traffic.
MM_HID_F32 = os.environ.get("BSDE_HID_F32", "0") == "1"

# how many h2 evacuations run on the scalar (ACT) engine instead of DVE,
# to balance the two engines' per-step load.
ACT_H2 = int(os.environ.get("BSDE_ACT_H2", "2"))

# offload the final-reduction accumulation ops (sqA product, accA/accP adds,
# p6 product) to the otherwise-idle GPSIMD engine.
GP_ACC = os.environ.get("BSDE_GP_ACC", "1") == "1"

# L1 matmuls in bf16 (stationary + a per-step bf16 copy of y): halves the
# f32r weight-load time on the PE at a tiny precision cost.
L1_BF16 = os.environ.get("BSDE_L1_BF16", "0") == "1"

# split the out3s evacuation: q rows via ACT feed a K=32 broadcast matmul so
# the z-row evacuation leaves the step-boundary critical path.
SPLIT_TAIL = os.environ.get("BSDE_SPLIT_TAIL", "0") == "1"

# emit PE work pair-adjacent (L1,L1,L2,L2,L3,L3 per chunk pair) so the two
# W2 loads sit back-to-back for weight-load pull-ahead.
PAIR_EMIT = os.environ.get("BSDE_PAIR_EMIT", "0") == "1"

# deeper activation rings to absorb evacuation jitter (SBUF has headroom)
RING_H = os.environ.get("BSDE_RING_H", "1") == "1"

# give ps2 a 4th bank by allocating pq from ps3's ring (p3's bank is free by
# the time the broadcast runs; the WAR dep is exactly the required ordering)
PS2_4 = os.environ.get("BSDE_PS2_4", "1") == "1"

# evacuate out3s on the ACT engine (Identity + vector bias): at the step tail
# ACT's queue is empty while DVE is still draining h2 evacuations, so the
# p3 -> out3s -> broadcast -> y-add chain starts sooner.
TAIL_ACT = os.environ.get("BSDE_TAIL_ACT", "1") == "1"

# emit the diffusion chain (tanh, t1, t2, t3) spread through the chunk loop
# so only the final y-add remains in the step tail's DVE queue.
T3_EARLY = os.environ.get("BSDE_T3_EARLY", "0") == "1"

# run the diffusion chain (t1,t2,t3) on the idle GPSIMD queue (split into
# single-ALU ops), with tanh emitted mid-loop on ACT: the chain then finishes
# mid-step instead of serializing at the end of DVE's FIFO, so the step tail
# is just broadcast + y-add.
CHAIN_GP = os.environ.get("BSDE_CHAIN_GP", "0") == "1"

# assign the ACT-side h2 evacuations to the LAST chunks instead of the first:
# ACT helps drain the end of the h2 stream right before the step tail.
ACT_H2_LATE = os.environ.get("BSDE_ACT_H2_LATE", "1") == "1"

# If set (by the timing harness), the device loop runs this many steps while
# all I/O shapes stay identical — lets wall-clock differencing isolate the
# per-step device time from RPC/transfer overhead.
LOOP_STEPS = None


def _np_f32(x):
    return np.ascontiguousarray(np.asarray(x, dtype=np.float32))


def prep_host(inputs):
    """Build all device-side arrays (numpy fp32) from the raw problem inputs."""
    i = {k: _np_f32(v) for k, v in inputs.items()}
    qW1, qb1 = i["qW1"], i["qb1"]
    qW2, qb2 = i["qW2"], i["qb2"]
    qW3, qb3 = i["qW3"], i["qb3"]
    zW1, zb1 = i["zW1"], i["zb1"]
    zW2, zb2 = i["zW2"], i["zb2"]
    zW3, zb3 = i["zW3"], i["zb3"]
    y0 = i["y0"]
    Y0 = float(i["Y0"].reshape(-1)[0])
    dW = i["dW"]

    W1cat = np.concatenate([qW1, zW1], axis=1)          # [4, 128]
    b1cat = np.concatenate([qb1, zb1])                  # [128]

    # L1 reads the packed y state directly: per-chunk sparse stationaries.
    # chunk k's component i lives at partition 32 + 32*i + k.
    W1S = np.zeros((CHUNKS, 128, 128), np.float32)
    for k in range(CHUNKS):
        for comp in range(3):
            W1S[k, 32 + 32 * comp + k, :] = W1cat[1 + comp, :]
    W1S = np.ascontiguousarray(W1S.transpose(1, 0, 2).reshape(128, CHUNKS * 128))

    # per-step bias for the L1 relu evacuation: c_n = t_n * W1cat[0] + b1cat
    ts = (np.arange(N_STEPS, dtype=np.float32) * np.float32(DT))
    CB = (ts[None, :] * W1cat[0][:, None] + b1cat[:, None]).astype(np.float32)  # [128, 50]

    W2 = np.zeros((128, 128), np.float32)
    W2[0:64, 0:64] = qW2
    W2[64:128, 64:128] = zW2
    B2 = b1cat * 0.0
    B2 = np.concatenate([qb2, zb2]).astype(np.float32).reshape(128, 1)

    # L3: per-chunk sparse stationaries [128, 32*128]
    W3S = np.zeros((CHUNKS, 128, 128), np.float32)
    for k in range(CHUNKS):
        W3S[k, 0:64, k] = DT * qW3[:, 0]
        for comp in range(3):
            W3S[k, 64:128, 32 + 32 * comp + k] = SQRT_DT * zW3[:, comp]
    W3S = np.ascontiguousarray(W3S.transpose(1, 0, 2).reshape(128, CHUNKS * 128))

    B3 = np.zeros((128, 1), np.float32)
    B3[0:32, 0] = DT * qb3[0]
    for comp in range(3):
        B3[32 + 32 * comp:64 + 32 * comp, 0] = SQRT_DT * zb3[comp]

    # broadcast matmul: qrep[32+32i+k] = out3s[k]
    BB = np.zeros((128, 128), np.float32)
    for k in range(CHUNKS):
        for comp in range(3):
            BB[k, 32 + 32 * comp + k] = 1.0

    # final reduction: col j sums the 3 components of chunk j
    TT = np.zeros((128, 32), np.float32)
    for j in range(CHUNKS):
        for comp in range(3):
            TT[32 + 32 * comp + j, j] = 1.0

    # initial y state, broadcast to full [128, 512] (q-slot rows zero)
    YINIT = np.zeros((128, FREE), np.float32)
    for comp in range(3):
        YINIT[32 + 32 * comp:64 + 32 * comp, :] = y0[comp]

    # per-core dW, transposed to [steps, comp, b_loc]; row block 0 (the q-slot
    # partitions) is zeros so a single full-tile DMA initializes everything
    dWt_cores = []
    for r in range(N_CORES):
        sl = dW[:N_STEPS, r * B_LOC:(r + 1) * B_LOC, :]     # [N_STEPS, B_loc, 3]
        t = np.zeros((N_STEPS, 4 * B_LOC), np.float32)
        t[:, B_LOC:] = sl.transpose(0, 2, 1).reshape(N_STEPS, 3 * B_LOC)
        dWt_cores.append(t)

    return dict(
        W1S=W1S, CB=CB, W2=W2, B2=B2, W3S=W3S, B3=B3, BB=BB, TT=TT,
        YINIT=YINIT, dWt_cores=dWt_cores, Y0=Y0,
    )


def _split_sync_waits(bir: dict) -> dict:
    """Walrus in this toolchain accepts only ~1 sync wait per instruction.
    Hoist extra waits onto standalone EventSemaphore instructions inserted
    just before, on the same engine (waits-only, so semantics unchanged)."""
    n = 0
    for fn in bir.get("functions", []):
        for bb in fn.get("blocks", []):
            out = []
            for ins in bb.get("instructions", []):
                si = ins.get("sync_info")
                waits = (si or {}).get("on_wait") or []
                if len(waits) > 1:
                    for w in waits[:-1]:
                        n += 1
                        out.append({
                            "engine": ins["engine"],
                            "ins": [],
                            "outs": [],
                            "name": f"bsdewait{n}_{ins['name']}",
                            "opcode": "EventSemaphore",
                            "debug": ins.get("debug", 0),
                            "sync_info": {"on_update": [], "on_wait": [w]},
                        })
                    si["on_wait"] = [waits[-1]]
                out.append(ins)
            bb["instructions"] = out
    return bir


def _install_ldw_opt():
    """walrus is invoked with --enable-ldw-opt=false; flip it on so repeated/
    adjacent stationary loads are optimized (gated by BSDE_LDW_OPT)."""
    from concourse import bass_utils
    if os.environ.get("BSDE_LDW_OPT", "0") != "1":
        return
    if getattr(bass_utils, "_bsde_ldwopt_installed", False):
        return
    orig = bass_utils.run_command

    def wrapped(cmd, **kw):
        if isinstance(cmd, list):
            cmd = ["--enable-ldw-opt=true" if c == "--enable-ldw-opt=false" else c
                   for c in cmd]
        return orig(cmd, **kw)

    bass_utils.run_command = wrapped
    bass_utils._bsde_ldwopt_installed = True


def _install_wait_splitter():
    import json as _json
    from concourse import bass2jax, bass_utils
    if getattr(bass_utils, "_bsde_split_installed", False):
        return
    orig = bass_utils.compile_bir_kernel

    def wrapped(bir_json, tmpdir, neff_name="file.neff"):
        bir = _json.loads(bir_json)
        _split_sync_waits(bir)
        return orig(_json.dumps(bir).encode(), tmpdir, neff_name)

    bass_utils.compile_bir_kernel = wrapped
    bass2jax.compile_bir_kernel = wrapped
    bass_utils._bsde_split_installed = True


def build_program():
    """Build the Bass program (same for all cores). Returns (nc, meta)."""
    from concourse import bass, mybir, tile

    f32 = mybir.dt.float32
    f32r = mybir.dt.float32r
    hdt = f32 if MM_HID_F32 else mybir.dt.bfloat16
    Alu = mybir.AluOpType
    Act = mybir.ActivationFunctionType

    def R(ap):
        # reinterpret fp32 data as float32r for full-rate PE streaming
        return ap.bitcast(f32r)

    def H(ap):
        # hidden-path operand: native bf16, or f32->f32r bitcast in fallback
        return R(ap) if hdt == f32 else ap

    nc = bass.Bass("TRN2", target_bir_lowering=False, debug=False)

    # --- dram I/O ---
    d_dWt = nc.dram_tensor("dWt", [N_STEPS, 4 * B_LOC], f32, kind="ExternalInput").ap()
    d_W1S = nc.dram_tensor("W1S", [128, CHUNKS * 128], f32r, kind="ExternalInput").ap()
    d_CB = nc.dram_tensor("CB", [128, N_STEPS], f32, kind="ExternalInput").ap()
    d_W2 = nc.dram_tensor("W2", [128, 128], f32, kind="ExternalInput").ap()
    d_B2 = nc.dram_tensor("B2", [128, 1], f32, kind="ExternalInput").ap()
    d_W3S = nc.dram_tensor("W3S", [128, CHUNKS * 128], f32, kind="ExternalInput").ap()
    d_B3 = nc.dram_tensor("B3", [128, 1], f32, kind="ExternalInput").ap()
    d_BB = nc.dram_tensor("BB", [128, 128], f32r, kind="ExternalInput").ap()
    d_TT = nc.dram_tensor("TT", [128, 32], f32r, kind="ExternalInput").ap()
    d_YI = nc.dram_tensor("YINIT", [128, FREE], f32r, kind="ExternalInput").ap()
    d_SC = nc.dram_tensor("SCAL", [4, 1], f32, kind="ExternalInput").ap()  # [Y0; -0.5/dt; a; b]
    d_res = nc.dram_tensor("res", [32, 1], f32, kind="ExternalOutput").ap()

    a_coef = 0.1 * SQRT_DT
    b_coef = 0.2 * SQRT_DT

    with tile.TileContext(nc) as tc:
        with (
            tc.tile_pool(name="consts", bufs=1) as consts,
            tc.tile_pool(name="state", bufs=1) as state,
            tc.tile_pool(name="h1p", bufs=(6 if RING_H else 3)) as h1pool,
            tc.tile_pool(name="h2p", bufs=(8 if RING_H else 4)) as h2pool,
            tc.tile_pool(name="tmp", bufs=1) as tmp,
            tc.tile_pool(name="dwp", bufs=8) as dwp,
            tc.tile_pool(name="ps1", bufs=3, space="PSUM") as ps1,
            tc.tile_pool(name="ps2", bufs=(4 if PS2_4 else 3), space="PSUM") as ps2,
            tc.tile_pool(name="ps3", bufs=1, space="PSUM") as ps3,
            tc.tile_pool(name="psq", bufs=1, space="PSUM") as psq,
        ):
            # ---- load constants into SBUF ----
            _dma_eng = [nc.sync, nc.scalar, nc.gpsimd]
            _dma_i = [0]

            def load_const(name, dram_ap, shape, dt_):
                t = consts.tile(shape, dt_, tag=name)
                eng = _dma_eng[_dma_i[0] % len(_dma_eng)]
                _dma_i[0] += 1
                eng.dma_start(t[:], dram_ap)
                return t

            W1S = load_const("W1S", d_W1S, [128, CHUNKS * 128], f32r)
            CB = load_const("CB", d_CB, [128, N_STEPS], f32)
            W2 = load_const("W2", d_W2, [128, 128], f32)
            B2 = load_const("B2", d_B2, [128, 1], f32)
            W3S = load_const("W3S", d_W3S, [128, CHUNKS * 128], f32)
            B3 = load_const("B3", d_B3, [128, 1], f32)
            BB = load_const("BB", d_BB, [128, 128], f32r)
            TT = load_const("TT", d_TT, [128, 32], f32r)

            if L1_BF16:
                W1Sb = consts.tile([128, CHUNKS * 128], mybir.dt.bfloat16,
                                   tag="W1Sb", name="W1Sb")
                nc.vector.tensor_copy(W1Sb[:], W1S[:])

            if hdt == f32:
                W2m, W3Sm = W2, W3S
            else:
                W2m = consts.tile([128, 128], hdt, tag="W2m", name="W2m")
                nc.vector.tensor_copy(W2m[:], W2[:])
                W3Sm = consts.tile([128, CHUNKS * 128], hdt, tag="W3Sm", name="W3Sm")
                nc.vector.tensor_copy(W3Sm[:], W3S[:])

            # ---- persistent state ----
            y_pl = state.tile([128, FREE], f32r, tag="y_pl", name="y_pl")
            nc.gpsimd.dma_start(y_pl[:], d_YI)
            accA = state.tile([32, FREE], f32, tag="accA", name="accA")
            nc.vector.memset(accA[:], 0.0)
            accP = state.tile([128, FREE], f32, tag="accP", name="accP")
            nc.vector.memset(accP[:], 0.0)
            out3s = state.tile([128, FREE], f32r, tag="out3s", name="out3s")
            if L1_BF16:
                y_plb = state.tile([128, FREE], mybir.dt.bfloat16,
                                   tag="y_plb", name="y_plb")

            # ---- time loop ----
            # Chunk-interleaved pipeline: per chunk L1 -> (ACT evac) -> L2 ->
            # (DVE/ACT evac) -> L3-accumulate; the psum rings provide the
            # cross-chunk overlap. SBUF-only elementwise work runs on the
            # otherwise-idle GPSIMD.
            n_loop = N_STEPS if LOOP_STEPS is None else LOOP_STEPS
            for n in range(n_loop):
                dw_t = dwp.tile([128, FREE], f32, tag="dw", name="dw")
                nc.gpsimd.dma_start(dw_t[:, :], d_dWt[n, :])

                p3 = ps3.tile([128, FREE], f32, tag="p3", name="p3")
                cb_n = CB[:, n:n + 1]

                if L1_BF16:
                    nc.gpsimd.tensor_copy(y_plb[:], y_pl[:])
                    l1_w, l1_y = W1Sb, y_plb
                else:
                    l1_w, l1_y = W1S, y_pl

                def emit_L1(k):
                    p1 = ps1.tile([128, FREE], f32, tag="p1", name="p1")
                    nc.tensor.matmul(p1[:], l1_w[:, k * 128:(k + 1) * 128], l1_y[:])
                    return p1

                def emit_evac1(p1):
                    h1 = h1pool.tile([128, FREE], hdt, tag="h1", name="h1")
                    nc.scalar.activation(h1[:], p1[:], Act.Relu, bias=cb_n)
                    return h1

                def emit_L2(h1):
                    p2 = ps2.tile([128, FREE], f32, tag="p2", name="p2")
                    nc.tensor.matmul(p2[:], H(W2m[:]), H(h1[:]))
                    return p2

                def emit_evac2(k, p2):
                    h2 = h2pool.tile([128, FREE], hdt, tag="h2", name="h2")
                    on_act = (k >= CHUNKS - ACT_H2) if ACT_H2_LATE else (k < ACT_H2)
                    if on_act:
                        nc.scalar.activation(h2[:], p2[:], Act.Relu, bias=B2[:, 0:1])
                    else:
                        nc.vector.tensor_scalar(h2[:], p2[:], B2[:, 0:1], 0.0,
                                                Alu.add, Alu.max)
                    return h2

                def emit_L3(k, h2):
                    nc.tensor.matmul(
                        p3[:], H(W3Sm[:, k * 128:(k + 1) * 128]), H(h2[:]),
                        start=(k == 0), stop=(k == CHUNKS - 1),
                    )

                if PAIR_EMIT:
                    for j in range(CHUNKS // 2):
                        ka, kb = 2 * j, 2 * j + 1
                        p1a = emit_L1(ka)
                        h1a = emit_evac1(p1a)
                        p1b = emit_L1(kb)
                        h1b = emit_evac1(p1b)
                        p2a = emit_L2(h1a)
                        p2b = emit_L2(h1b)
                        h2a = emit_evac2(ka, p2a)
                        h2b = emit_evac2(kb, p2b)
                        emit_L3(ka, h2a)
                        emit_L3(kb, h2b)
                else:
                    for k in range(CHUNKS):
                        if CHAIN_GP and k == 10:
                            th = tmp.tile([128, FREE], f32, tag="th", name="th")
                            nc.scalar.activation(th[:], y_pl[:], Act.Tanh)
                            t1 = tmp.tile([128, FREE], f32, tag="t1", name="t1")
                            nc.gpsimd.tensor_scalar_mul(t1[:], th[:], a_coef)
                            nc.gpsimd.tensor_scalar_add(t1[:], t1[:], b_coef)
                            t2 = tmp.tile([128, FREE], f32, tag="t2", name="t2")
                            nc.gpsimd.tensor_tensor(t2[:], t1[:], dw_t[:], Alu.mult)
                            t3 = tmp.tile([128, FREE], f32, tag="t3", name="t3")
                            nc.gpsimd.tensor_scalar_mul(t3[:], y_pl[:], 1.0 - DT)
                            nc.gpsimd.tensor_tensor(t3[:], t3[:], t2[:], Alu.add)
                        if T3_EARLY:
                            if k == 10:
                                th = tmp.tile([128, FREE], f32, tag="th", name="th")
                                nc.scalar.activation(th[:], y_pl[:], Act.Tanh)
                            elif k == 14:
                                t1 = tmp.tile([128, FREE], f32, tag="t1", name="t1")
                                nc.vector.tensor_scalar(t1[:], th[:], a_coef, b_coef,
                                                        Alu.mult, Alu.add)
                            elif k == 18:
                                t2 = tmp.tile([128, FREE], f32, tag="t2", name="t2")
                                nc.gpsimd.tensor_tensor(t2[:], t1[:], dw_t[:], Alu.mult)
                            elif k == 24:
                                t3 = tmp.tile([128, FREE], f32, tag="t3", name="t3")
                                nc.vector.scalar_tensor_tensor(t3[:], y_pl[:], 1.0 - DT,
                                                               t2[:], Alu.mult, Alu.add)
                        p1 = emit_L1(k)
                        h1 = emit_evac1(p1)
                        p2 = emit_L2(h1)
                        h2 = emit_evac2(k, p2)
                        emit_L3(k, h2)

                if SPLIT_TAIL:
                    # q rows evacuate first (ACT) and alone feed the K=32
                    # broadcast; z rows evacuate on DVE off the critical path
                    oq = tmp.tile([32, FREE], f32r, tag="oq", name="oq")
                    nc.vector.tensor_scalar(oq[:], p3[0:32, :], B3[0:32, 0:1], None, Alu.add)
                    pqpool, pqtag = (ps3, "p3") if PS2_4 else (psq, "pq")
                    pq = pqpool.tile([128, FREE], f32, tag=pqtag, name="pq")
                    nc.tensor.matmul(pq[:], BB[0:32, :], oq[:])
                    nc.vector.tensor_scalar(out3s[32:128, :], p3[32:128, :],
                                            B3[32:128, 0:1], None, Alu.add)
                else:
                    # out3s = psum3 + per-partition bias
                    if TAIL_ACT:
                        nc.scalar.activation(out3s[:], p3[:], Act.Identity, bias=B3[:, 0:1])
                    else:
                        nc.vector.tensor_scalar(out3s[:], p3[:], B3[:, 0:1], None, Alu.add)

                    # qrep = broadcast dt*q to component quarters (via PE)
                    pqpool, pqtag = (ps3, "p3") if PS2_4 else (psq, "pq")
                    pq = pqpool.tile([128, FREE], f32, tag=pqtag, name="pq")
                    nc.tensor.matmul(pq[:], BB[:], out3s[:])

                # ---- elementwise state update ----
                if not T3_EARLY and not CHAIN_GP:
                    th = tmp.tile([128, FREE], f32, tag="th", name="th")
                    nc.scalar.activation(th[:], y_pl[:], Act.Tanh)
                    t1 = tmp.tile([128, FREE], f32, tag="t1", name="t1")
                    nc.vector.tensor_scalar(t1[:], th[:], a_coef, b_coef, Alu.mult, Alu.add)
                    t2 = tmp.tile([128, FREE], f32, tag="t2", name="t2")
                    nc.vector.scalar_tensor_tensor(t2[:], t1[:], 1.0, dw_t[:], Alu.mult, Alu.mult)
                    t3 = tmp.tile([128, FREE], f32, tag="t3", name="t3")
                    nc.vector.scalar_tensor_tensor(t3[:], y_pl[:], 1.0 - DT, t2[:], Alu.mult, Alu.add)
                nc.vector.scalar_tensor_tensor(y_pl[:], t3[:], 0.0, pq[:], Alu.add, Alu.add)

                sqA = tmp.tile([32, FREE], f32, tag="sqA", name="sqA")
                p6 = tmp.tile([128, FREE], f32, tag="p6", name="p6")
                if SPLIT_TAIL:
                    nc.gpsimd.tensor_tensor(sqA[:], oq[:], oq[:], Alu.mult)
                    nc.gpsimd.tensor_tensor(accA[:], accA[:], sqA[:], Alu.add)
                    nc.gpsimd.tensor_tensor(p6[32:128, :], out3s[32:128, :],
                                            dw_t[32:128, :], Alu.mult)
                    nc.gpsimd.tensor_tensor(accP[32:128, :], accP[32:128, :],
                                            p6[32:128, :], Alu.add)
                elif GP_ACC:
                    nc.gpsimd.tensor_tensor(sqA[:], out3s[0:32, :], out3s[0:32, :], Alu.mult)
                    nc.gpsimd.tensor_tensor(accA[:], accA[:], sqA[:], Alu.add)
                    nc.gpsimd.tensor_tensor(p6[:], out3s[:], dw_t[:], Alu.mult)
                    nc.gpsimd.tensor_tensor(accP[:], accP[:], p6[:], Alu.add)
                else:
                    nc.scalar.activation(sqA[:], out3s[0:32, :], Act.Square)
                    nc.vector.scalar_tensor_tensor(accA[:], accA[:], 0.0, sqA[:], Alu.add, Alu.add)
                    nc.vector.scalar_tensor_tensor(p6[:], out3s[:], 1.0, dw_t[:], Alu.mult, Alu.mult)
                    nc.vector.scalar_tensor_tensor(accP[:], accP[:], 0.0, p6[:], Alu.add, Alu.add)

            # ---- final loss assembly ----
            ysq = tmp.tile([128, FREE], f32r, tag="ysq", name="ysq")
            nc.scalar.activation(ysq[:], y_pl[:], Act.Square)
            p_term = ps1.tile([32, FREE], f32, tag="p1", name="pterm")
            nc.tensor.matmul(p_term[:], TT[:], ysq[:])
            accPr = tmp.tile([128, FREE], f32r, tag="accPr", name="accPr")
            nc.vector.tensor_copy(accPr[:], accP[:])
            p_P = ps2.tile([32, FREE], f32, tag="p2", name="pP")
            nc.tensor.matmul(p_P[:], TT[:], accPr[:])

            Pg = tmp.tile([32, FREE], f32, tag="Pg", name="Pg")
            nc.vector.tensor_scalar(Pg[:], p_P[:], 0.0, None, Alu.add)
            Tg = tmp.tile([32, FREE], f32, tag="Tg", name="Tg")
            nc.vector.tensor_scalar(Tg[:], p_term[:], 0.0, None, Alu.add)
            D1 = tmp.tile([32, FREE], f32, tag="D1", name="D1")
            nc.vector.scalar_tensor_tensor(D1[:], accA[:], -0.5 / DT, Pg[:], Alu.mult, Alu.add)
            D2 = tmp.tile([32, FREE], f32, tag="D2", name="D2")
            nc.vector.scalar_tensor_tensor(D2[:], Tg[:], -1.0, D1[:], Alu.mult, Alu.add)
            # add Y0 (runtime input, broadcast from SCAL[0])
            sc = consts.tile([4, 1], f32, tag="SCAL", name="SCAL")
            nc.gpsimd.dma_start(sc[:], d_SC)
            y0b = consts.tile([32, 1], f32, tag="y0b", name="y0b")
            nc.gpsimd.dma_start(y0b[:], bass.AP(tensor=d_SC.tensor, offset=0, ap=[[0, 32], [1, 1]]))
            D3 = tmp.tile([32, FREE], f32, tag="D3", name="D3")
            nc.vector.tensor_scalar(D3[:], D2[:], y0b[:, 0:1], None, Alu.add)

            dsq = tmp.tile([32, FREE], f32, tag="dsq", name="dsq")
            res = state.tile([32, 1], f32, tag="res", name="res")
            nc.scalar.activation(dsq[:], D3[:], Act.Square, accum_out=res[:])
            nc.sync.dma_start(d_res, res[:])

    return nc


LAST_EXEC_NS = None
LAST_TRACE_DIR = None


def kernel(**inputs) -> np.ndarray:
    global LAST_EXEC_NS, LAST_TRACE_DIR
    from concourse.bass_utils import run_bass_kernel_spmd
    _install_wait_splitter()
    _install_ldw_opt()

    host = prep_host(inputs)

    nc = build_program()

    scal = np.array([[host["Y0"]], [-0.5 / DT], [0.1 * SQRT_DT], [0.2 * SQRT_DT]], np.float32)
    shared = dict(
        W1S=host["W1S"], CB=host["CB"], W2=host["W2"], B2=host["B2"],
        W3S=host["W3S"], B3=host["B3"], BB=host["BB"], TT=host["TT"],
        YINIT=host["YINIT"], SCAL=scal,
    )
    in_maps = []
    for r in range(N_CORES):
        m = dict(shared)
        m["dWt"] = host["dWt_cores"][r]
        in_maps.append(m)

    trace = os.environ.get("BSDE_TRACE", "0") == "1"
    kw = {}
    if trace:
        kw["trace"] = True
        kw["tmpdir"] = os.environ.get("BSDE_TRACE_DIR") or None
    out = run_bass_kernel_spmd(nc, in_maps, list(range(N_CORES)), **kw)
    LAST_EXEC_NS = getattr(out, "exec_time_ns", None)
    total = np.float64(0.0)
    for r in range(N_CORES):
        total += np.sum(out.results[r]["res"].astype(np.float64))
    return np.float32(total / BATCH)


def _build_in_maps(host):
    scal = np.array([[host["Y0"]], [-0.5 / DT], [0.1 * SQRT_DT], [0.2 * SQRT_DT]], np.float32)
    shared = dict(
        W1S=host["W1S"], CB=host["CB"], W2=host["W2"], B2=host["B2"],
        W3S=host["W3S"], B3=host["B3"], BB=host["BB"], TT=host["TT"],
        YINIT=host["YINIT"], SCAL=scal,
    )
    in_maps = []
    for r in range(N_CORES):
        m = dict(shared)
        m["dWt"] = host["dWt_cores"][r]
        in_maps.append(m)
    return in_maps


def timed_run(nc, in_maps, iters=7):
    """Mirror bass2jax.run_bass_via_pjrt's multi-core path, but keep inputs
    device-resident and time steady-state executions. Returns (results_core0,
    sorted wall times in ns per call)."""
    import time
    import jax
    from jax.sharding import Mesh, PartitionSpec, NamedSharding
    from jax.experimental.shard_map import shard_map
    from concourse import bass2jax, mybir

    bass2jax.install_neuronx_cc_hook()
    n_cores = N_CORES

    in_names, out_names, out_avals, zero_outs = [], [], [], []
    for alloc in nc.m.functions[0].allocations:
        if not isinstance(alloc, mybir.MemoryLocationSet):
            continue
        name = alloc.memorylocations[0].name
        if alloc.kind == "ExternalInput":
            in_names.append(name)
        elif alloc.kind == "ExternalOutput":
            out_names.append(name)
            shape = tuple(alloc.tensor_shape)
            dtype = mybir.dt.np(alloc.dtype)
            out_avals.append(jax.core.ShapedArray(shape, dtype))
            zero_outs.append(np.zeros(shape, dtype))
    n_params = len(in_names)
    n_outs = len(out_avals)
    all_names = in_names + out_names
    donate = tuple(range(n_params, n_params + n_outs))

    def _body(*args):
        outs = bass2jax._bass_exec_p.bind(
            *list(args),
            out_avals=tuple(out_avals),
            in_names=tuple(all_names),
            out_names=tuple(out_names),
            lowering_input_output_aliases=(),
            sim_require_finite=True,
            sim_require_nnan=True,
            nc=nc,
        )
        return tuple(outs)

    devices = jax.devices()[:n_cores]
    mesh = Mesh(np.asarray(devices), ("core",))
    in_specs = (PartitionSpec("core"),) * (n_params + n_outs)
    out_specs = (PartitionSpec("core"),) * len(out_names)
    sharded = jax.jit(
        shard_map(_body, mesh=mesh, in_specs=in_specs, out_specs=out_specs, check_rep=False),
        donate_argnums=donate,
        keep_unused=True,
    )
    concat_in = [
        np.concatenate([np.asarray(in_maps[c][nm]) for c in range(n_cores)], axis=0)
        for nm in in_names
    ]
    sh = NamedSharding(mesh, PartitionSpec("core"))
    dev_in = [jax.device_put(a, sh) for a in concat_in]
    concat_zeros = [np.zeros((n_cores * z.shape[0], *z.shape[1:]), z.dtype) for z in zero_outs]

    out = sharded(*dev_in, *concat_zeros)   # warm-up / compile
    jax.block_until_ready(out)
    times = []
    for _ in range(iters):
        zz = [np.zeros((n_cores * z.shape[0], *z.shape[1:]), z.dtype) for z in zero_outs]
        t0 = time.perf_counter_ns()
        out = sharded(*dev_in, *zz)
        jax.block_until_ready(out)
        times.append(time.perf_counter_ns() - t0)
    res0 = {
        nm: np.asarray(out[i]).reshape(n_cores, *out_avals[i].shape)
        for i, nm in enumerate(out_names)
    }
    return res0, sorted(times)


if __name__ == "__main__":
    rng = np.random.default_rng(0)
    fake = {
        "y0": rng.standard_normal(3).astype(np.float32),
        "Y0": np.zeros((1, 1), np.float32),
        "qW1": rng.standard_normal((4, 64)).astype(np.float32) * 0.5,
        "qb1": np.zeros(64, np.float32),
        "qW2": rng.standard_normal((64, 64)).astype(np.float32) * 0.12,
        "qb2": np.zeros(64, np.float32),
        "qW3": rng.standard_normal((64, 1)).astype(np.float32) * 0.12,
        "qb3": np.zeros(1, np.float32),
        "zW1": rng.standard_normal((4, 64)).astype(np.float32) * 0.5,
        "zb1": np.zeros(64, np.float32),
        "zW2": rng.standard_normal((64, 64)).astype(np.float32) * 0.12,
        "zb2": np.zeros(64, np.float32),
        "zW3": rng.standard_normal((64, 3)).astype(np.float32) * 0.12,
        "zb3": np.zeros(3, np.float32),
        "dW": rng.standard_normal((N_STEPS, BATCH, 3)).astype(np.float32),
    }
    print(kernel(**fake))



# revision 36
# speedup vs baseline: 1.2516x; 1.0340x over previous
"""DeepBSDE forward-loss kernel for Trainium2 (8 NeuronCores, data-parallel).

Math (per sample b, 50 steps, dt=0.02):
    x_n = [t_n, y_n]                       (4 features)
    z_n = MLP_z(x_n)   (4->64->64->3, relu)
    q_n = MLP_q(x_n)   (4->64->64->1, relu)
    y_{n+1} = (1-dt) y_n + dt q_n + (0.2 + 0.1 tanh(y_n)) * sqrt(dt) * dW_n
    Y_final = Y0 - 0.5 dt sum_n q_n^2 + sum_n z_n . (sqrt(dt) dW_n)
    out = mean_b (Y_final - |y_final|^2)^2

Device layout (per core, B_loc = 16384 = 32 chunks x 512):
    every per-sample state lives in a [128, 512] SBUF tile:
      partition k        (k in 0..31)   : q-slot of chunk k
      partition 32+32i+k (i in 0..2)    : vector component i of chunk k
      free c                            : sample index b = k*512 + c
    The two MLPs are fused: hidden = [q-hidden(64) ; z-hidden(64)] = 128.
    Per step the PE streams: L1 (K=3), L2 (K=128), L3 (per-chunk sparse
    [128,128] stationaries accumulated into ONE packed psum bank), plus one
    broadcast matmul that replicates dt*q into the 3 component quarters.
"""

import sys
import os

for _p in ("/opt/trn_rl_repo", "/root/.axon_site/_ro/trn_rl_repo"):
    if os.path.isdir(_p) and _p not in sys.path:
        sys.path.insert(0, _p)

import numpy as np

DT = 0.02
SQRT_DT = float(np.sqrt(np.float32(DT)))
N_STEPS = 50
BATCH = 131072
DIM = 3
N_CORES = 8
B_LOC = BATCH // N_CORES          # 16384
CHUNKS = 32
FREE = B_LOC // CHUNKS            # 512

# dtype knobs.
#  - L1/BB/TT matmuls read fp32 state; run them as float32r (same 4-byte
#    storage, 1 cycle/row on the PE at moving size >= 256 vs 4 for fp32).
#  - hidden activations h1/h2 and the L2/L3 weights run in bf16: same PE
#    rate as f32r but half the ACT/DVE evacuation cost and half the
#    weight-load traffic.
MM_HID_F32 = os.environ.get("BSDE_HID_F32", "0") == "1"

# how many h2 evacuations run on the scalar (ACT) engine instead of DVE,
# to balance the two engines' per-step load.
ACT_H2 = int(os.environ.get("BSDE_ACT_H2", "2"))

# offload the final-reduction accumulation ops (sqA product, accA/accP adds,
# p6 product) to the otherwise-idle GPSIMD engine.
GP_ACC = os.environ.get("BSDE_GP_ACC", "1") == "1"

# L1 matmuls in bf16 (stationary + a per-step bf16 copy of y): halves the
# f32r weight-load time on the PE at a tiny precision cost.
L1_BF16 = os.environ.get("BSDE_L1_BF16", "0") == "1"

# split the out3s evacuation: q rows via ACT feed a K=32 broadcast matmul so
# the z-row evacuation leaves the step-boundary critical path.
SPLIT_TAIL = os.environ.get("BSDE_SPLIT_TAIL", "0") == "1"

# emit PE work pair-adjacent (L1,L1,L2,L2,L3,L3 per chunk pair) so the two
# W2 loads sit back-to-back for weight-load pull-ahead.
PAIR_EMIT = os.environ.get("BSDE_PAIR_EMIT", "0") == "1"

# deeper activation rings to absorb evacuation jitter (SBUF has headroom)
RING_H = os.environ.get("BSDE_RING_H", "1") == "1"

# give ps2 a 4th bank by allocating pq from ps3's ring (p3's bank is free by
# the time the broadcast runs; the WAR dep is exactly the required ordering)
PS2_4 = os.environ.get("BSDE_PS2_4", "1") == "1"

# evacuate out3s on the ACT engine (Identity + vector bias): at the step tail
# ACT's queue is empty while DVE is still draining h2 evacuations, so the
# p3 -> out3s -> broadcast -> y-add chain starts sooner.
TAIL_ACT = os.environ.get("BSDE_TAIL_ACT", "1") == "1"

# emit the diffusion chain (tanh, t1, t2, t3) spread through the chunk loop
# so only the final y-add remains in the step tail's DVE queue.
T3_EARLY = os.environ.get("BSDE_T3_EARLY", "0") == "1"

# run the diffusion chain (t1,t2,t3) on the idle GPSIMD queue (split into
# single-ALU ops), with tanh emitted mid-loop on ACT: the chain then finishes
# mid-step instead of serializing at the end of DVE's FIFO, so the step tail
# is just broadcast + y-add.
CHAIN_GP = os.environ.get("BSDE_CHAIN_GP", "0") == "1"

# assign the ACT-side h2 evacuations to the LAST chunks instead of the first:
# ACT helps drain the end of the h2 stream right before the step tail.
ACT_H2_LATE = os.environ.get("BSDE_ACT_H2_LATE", "1") == "1"

# number of trailing h1 evacuations moved from ACT to DVE so ACT reaches the
# out3s evacuation (start of the step-tail chain) sooner.
DVE_H1 = int(os.environ.get("BSDE_DVE_H1", "0"))

# number of LEADING h1 evacuations moved from ACT to DVE: at step start DVE
# is idle (no h2 backlog yet), so this sheds ACT load without delaying the PE.
DVE_H1_EARLY = int(os.environ.get("BSDE_DVE_H1_EARLY", "2"))

# If set (by the timing harness), the device loop runs this many steps while
# all I/O shapes stay identical — lets wall-clock differencing isolate the
# per-step device time from RPC/transfer overhead.
LOOP_STEPS = None


def _np_f32(x):
    return np.ascontiguousarray(np.asarray(x, dtype=np.float32))


def prep_host(inputs):
    """Build all device-side arrays (numpy fp32) from the raw problem inputs."""
    i = {k: _np_f32(v) for k, v in inputs.items()}
    qW1, qb1 = i["qW1"], i["qb1"]
    qW2, qb2 = i["qW2"], i["qb2"]
    qW3, qb3 = i["qW3"], i["qb3"]
    zW1, zb1 = i["zW1"], i["zb1"]
    zW2, zb2 = i["zW2"], i["zb2"]
    zW3, zb3 = i["zW3"], i["zb3"]
    y0 = i["y0"]
    Y0 = float(i["Y0"].reshape(-1)[0])
    dW = i["dW"]

    W1cat = np.concatenate([qW1, zW1], axis=1)          # [4, 128]
    b1cat = np.concatenate([qb1, zb1])                  # [128]

    # L1 reads the packed y state directly: per-chunk sparse stationaries.
    # chunk k's component i lives at partition 32 + 32*i + k.
    W1S = np.zeros((CHUNKS, 128, 128), np.float32)
    for k in range(CHUNKS):
        for comp in range(3):
            W1S[k, 32 + 32 * comp + k, :] = W1cat[1 + comp, :]
    W1S = np.ascontiguousarray(W1S.transpose(1, 0, 2).reshape(128, CHUNKS * 128))

    # per-step bias for the L1 relu evacuation: c_n = t_n * W1cat[0] + b1cat
    ts = (np.arange(N_STEPS, dtype=np.float32) * np.float32(DT))
    CB = (ts[None, :] * W1cat[0][:, None] + b1cat[:, None]).astype(np.float32)  # [128, 50]

    W2 = np.zeros((128, 128), np.float32)
    W2[0:64, 0:64] = qW2
    W2[64:128, 64:128] = zW2
    B2 = b1cat * 0.0
    B2 = np.concatenate([qb2, zb2]).astype(np.float32).reshape(128, 1)

    # L3: per-chunk sparse stationaries [128, 32*128]
    W3S = np.zeros((CHUNKS, 128, 128), np.float32)
    for k in range(CHUNKS):
        W3S[k, 0:64, k] = DT * qW3[:, 0]
        for comp in range(3):
            W3S[k, 64:128, 32 + 32 * comp + k] = SQRT_DT * zW3[:, comp]
    W3S = np.ascontiguousarray(W3S.transpose(1, 0, 2).reshape(128, CHUNKS * 128))

    B3 = np.zeros((128, 1), np.float32)
    B3[0:32, 0] = DT * qb3[0]
    for comp in range(3):
        B3[32 + 32 * comp:64 + 32 * comp, 0] = SQRT_DT * zb3[comp]

    # broadcast matmul: qrep[32+32i+k] = out3s[k]
    BB = np.zeros((128, 128), np.float32)
    for k in range(CHUNKS):
        for comp in range(3):
            BB[k, 32 + 32 * comp + k] = 1.0

    # final reduction: col j sums the 3 components of chunk j
    TT = np.zeros((128, 32), np.float32)
    for j in range(CHUNKS):
        for comp in range(3):
            TT[32 + 32 * comp + j, j] = 1.0

    # initial y state, broadcast to full [128, 512] (q-slot rows zero)
    YINIT = np.zeros((128, FREE), np.float32)
    for comp in range(3):
        YINIT[32 + 32 * comp:64 + 32 * comp, :] = y0[comp]

    # per-core dW, transposed to [steps, comp, b_loc]; row block 0 (the q-slot
    # partitions) is zeros so a single full-tile DMA initializes everything
    dWt_cores = []
    for r in range(N_CORES):
        sl = dW[:N_STEPS, r * B_LOC:(r + 1) * B_LOC, :]     # [N_STEPS, B_loc, 3]
        t = np.zeros((N_STEPS, 4 * B_LOC), np.float32)
        t[:, B_LOC:] = sl.transpose(0, 2, 1).reshape(N_STEPS, 3 * B_LOC)
        dWt_cores.append(t)

    return dict(
        W1S=W1S, CB=CB, W2=W2, B2=B2, W3S=W3S, B3=B3, BB=BB, TT=TT,
        YINIT=YINIT, dWt_cores=dWt_cores, Y0=Y0,
    )


def _split_sync_waits(bir: dict) -> dict:
    """Walrus in this toolchain accepts only ~1 sync wait per instruction.
    Hoist extra waits onto standalone EventSemaphore instructions inserted
    just before, on the same engine (waits-only, so semantics unchanged)."""
    n = 0
    for fn in bir.get("functions", []):
        for bb in fn.get("blocks", []):
            out = []
            for ins in bb.get("instructions", []):
                si = ins.get("sync_info")
                waits = (si or {}).get("on_wait") or []
                if len(waits) > 1:
                    for w in waits[:-1]:
                        n += 1
                        out.append({
                            "engine": ins["engine"],
                            "ins": [],
                            "outs": [],
                            "name": f"bsdewait{n}_{ins['name']}",
                            "opcode": "EventSemaphore",
                            "debug": ins.get("debug", 0),
                            "sync_info": {"on_update": [], "on_wait": [w]},
                        })
                    si["on_wait"] = [waits[-1]]
                out.append(ins)
            bb["instructions"] = out
    return bir


def _install_ldw_opt():
    """walrus is invoked with --enable-ldw-opt=false; flip it on so repeated/
    adjacent stationary loads are optimized (gated by BSDE_LDW_OPT)."""
    from concourse import bass_utils
    if os.environ.get("BSDE_LDW_OPT", "0") != "1":
        return
    if getattr(bass_utils, "_bsde_ldwopt_installed", False):
        return
    orig = bass_utils.run_command

    def wrapped(cmd, **kw):
        if isinstance(cmd, list):
            cmd = ["--enable-ldw-opt=true" if c == "--enable-ldw-opt=false" else c
                   for c in cmd]
        return orig(cmd, **kw)

    bass_utils.run_command = wrapped
    bass_utils._bsde_ldwopt_installed = True


def _install_wait_splitter():
    import json as _json
    from concourse import bass2jax, bass_utils
    if getattr(bass_utils, "_bsde_split_installed", False):
        return
    orig = bass_utils.compile_bir_kernel

    def wrapped(bir_json, tmpdir, neff_name="file.neff"):
        bir = _json.loads(bir_json)
        _split_sync_waits(bir)
        return orig(_json.dumps(bir).encode(), tmpdir, neff_name)

    bass_utils.compile_bir_kernel = wrapped
    bass2jax.compile_bir_kernel = wrapped
    bass_utils._bsde_split_installed = True


def build_program():
    """Build the Bass program (same for all cores). Returns (nc, meta)."""
    from concourse import bass, mybir, tile

    f32 = mybir.dt.float32
    f32r = mybir.dt.float32r
    hdt = f32 if MM_HID_F32 else mybir.dt.bfloat16
    Alu = mybir.AluOpType
    Act = mybir.ActivationFunctionType

    def R(ap):
        # reinterpret fp32 data as float32r for full-rate PE streaming
        return ap.bitcast(f32r)

    def H(ap):
        # hidden-path operand: native bf16, or f32->f32r bitcast in fallback
        return R(ap) if hdt == f32 else ap

    nc = bass.Bass("TRN2", target_bir_lowering=False, debug=False)

    # --- dram I/O ---
    d_dWt = nc.dram_tensor("dWt", [N_STEPS, 4 * B_LOC], f32, kind="ExternalInput").ap()
    d_W1S = nc.dram_tensor("W1S", [128, CHUNKS * 128], f32r, kind="ExternalInput").ap()
    d_CB = nc.dram_tensor("CB", [128, N_STEPS], f32, kind="ExternalInput").ap()
    d_W2 = nc.dram_tensor("W2", [128, 128], f32, kind="ExternalInput").ap()
    d_B2 = nc.dram_tensor("B2", [128, 1], f32, kind="ExternalInput").ap()
    d_W3S = nc.dram_tensor("W3S", [128, CHUNKS * 128], f32, kind="ExternalInput").ap()
    d_B3 = nc.dram_tensor("B3", [128, 1], f32, kind="ExternalInput").ap()
    d_BB = nc.dram_tensor("BB", [128, 128], f32r, kind="ExternalInput").ap()
    d_TT = nc.dram_tensor("TT", [128, 32], f32r, kind="ExternalInput").ap()
    d_YI = nc.dram_tensor("YINIT", [128, FREE], f32r, kind="ExternalInput").ap()
    d_SC = nc.dram_tensor("SCAL", [4, 1], f32, kind="ExternalInput").ap()  # [Y0; -0.5/dt; a; b]
    d_res = nc.dram_tensor("res", [32, 1], f32, kind="ExternalOutput").ap()

    a_coef = 0.1 * SQRT_DT
    b_coef = 0.2 * SQRT_DT

    with tile.TileContext(nc) as tc:
        with (
            tc.tile_pool(name="consts", bufs=1) as consts,
            tc.tile_pool(name="state", bufs=1) as state,
            tc.tile_pool(name="h1p", bufs=(6 if RING_H else 3)) as h1pool,
            tc.tile_pool(name="h2p", bufs=(8 if RING_H else 4)) as h2pool,
            tc.tile_pool(name="tmp", bufs=1) as tmp,
            tc.tile_pool(name="dwp", bufs=8) as dwp,
            tc.tile_pool(name="ps1", bufs=3, space="PSUM") as ps1,
            tc.tile_pool(name="ps2", bufs=(4 if PS2_4 else 3), space="PSUM") as ps2,
            tc.tile_pool(name="ps3", bufs=1, space="PSUM") as ps3,
            tc.tile_pool(name="psq", bufs=1, space="PSUM") as psq,
        ):
            # ---- load constants into SBUF ----
            _dma_eng = [nc.sync, nc.scalar, nc.gpsimd]
            _dma_i = [0]

            def load_const(name, dram_ap, shape, dt_):
                t = consts.tile(shape, dt_, tag=name)
                eng = _dma_eng[_dma_i[0] % len(_dma_eng)]
                _dma_i[0] += 1
                eng.dma_start(t[:], dram_ap)
                return t

            W1S = load_const("W1S", d_W1S, [128, CHUNKS * 128], f32r)
            CB = load_const("CB", d_CB, [128, N_STEPS], f32)
            W2 = load_const("W2", d_W2, [128, 128], f32)
            B2 = load_const("B2", d_B2, [128, 1], f32)
            W3S = load_const("W3S", d_W3S, [128, CHUNKS * 128], f32)
            B3 = load_const("B3", d_B3, [128, 1], f32)
            BB = load_const("BB", d_BB, [128, 128], f32r)
            TT = load_const("TT", d_TT, [128, 32], f32r)

            if L1_BF16:
                W1Sb = consts.tile([128, CHUNKS * 128], mybir.dt.bfloat16,
                                   tag="W1Sb", name="W1Sb")
                nc.vector.tensor_copy(W1Sb[:], W1S[:])

            if hdt == f32:
                W2m, W3Sm = W2, W3S
            else:
                W2m = consts.tile([128, 128], hdt, tag="W2m", name="W2m")
                nc.vector.tensor_copy(W2m[:], W2[:])
                W3Sm = consts.tile([128, CHUNKS * 128], hdt, tag="W3Sm", name="W3Sm")
                nc.vector.tensor_copy(W3Sm[:], W3S[:])

            # ---- persistent state ----
            y_pl = state.tile([128, FREE], f32r, tag="y_pl", name="y_pl")
            nc.gpsimd.dma_start(y_pl[:], d_YI)
            accA = state.tile([32, FREE], f32, tag="accA", name="accA")
            nc.vector.memset(accA[:], 0.0)
            accP = state.tile([128, FREE], f32, tag="accP", name="accP")
            nc.vector.memset(accP[:], 0.0)
            out3s = state.tile([128, FREE], f32r, tag="out3s", name="out3s")
            if L1_BF16:
                y_plb = state.tile([128, FREE], mybir.dt.bfloat16,
                                   tag="y_plb", name="y_plb")

            # ---- time loop ----
            # Chunk-interleaved pipeline: per chunk L1 -> (ACT evac) -> L2 ->
            # (DVE/ACT evac) -> L3-accumulate; the psum rings provide the
            # cross-chunk overlap. SBUF-only elementwise work runs on the
            # otherwise-idle GPSIMD.
            n_loop = N_STEPS if LOOP_STEPS is None else LOOP_STEPS
            for n in range(n_loop):
                dw_t = dwp.tile([128, FREE], f32, tag="dw", name="dw")
                nc.gpsimd.dma_start(dw_t[:, :], d_dWt[n, :])

                p3 = ps3.tile([128, FREE], f32, tag="p3", name="p3")
                cb_n = CB[:, n:n + 1]

                if L1_BF16:
                    nc.gpsimd.tensor_copy(y_plb[:], y_pl[:])
                    l1_w, l1_y = W1Sb, y_plb
                else:
                    l1_w, l1_y = W1S, y_pl

                def emit_L1(k):
                    p1 = ps1.tile([128, FREE], f32, tag="p1", name="p1")
                    nc.tensor.matmul(p1[:], l1_w[:, k * 128:(k + 1) * 128], l1_y[:])
                    return p1

                def emit_evac1(k, p1):
                    h1 = h1pool.tile([128, FREE], hdt, tag="h1", name="h1")
                    if (DVE_H1 and k >= CHUNKS - DVE_H1) or (DVE_H1_EARLY and k < DVE_H1_EARLY):
                        nc.vector.tensor_scalar(h1[:], p1[:], cb_n, 0.0,
                                                Alu.add, Alu.max)
                    else:
                        nc.scalar.activation(h1[:], p1[:], Act.Relu, bias=cb_n)
                    return h1

                def emit_L2(h1):
                    p2 = ps2.tile([128, FREE], f32, tag="p2", name="p2")
                    nc.tensor.matmul(p2[:], H(W2m[:]), H(h1[:]))
                    return p2

                def emit_evac2(k, p2):
                    h2 = h2pool.tile([128, FREE], hdt, tag="h2", name="h2")
                    on_act = (k >= CHUNKS - ACT_H2) if ACT_H2_LATE else (k < ACT_H2)
                    if on_act:
                        nc.scalar.activation(h2[:], p2[:], Act.Relu, bias=B2[:, 0:1])
                    else:
                        nc.vector.tensor_scalar(h2[:], p2[:], B2[:, 0:1], 0.0,
                                                Alu.add, Alu.max)
                    return h2

                def emit_L3(k, h2):
                    nc.tensor.matmul(
                        p3[:], H(W3Sm[:, k * 128:(k + 1) * 128]), H(h2[:]),
                        start=(k == 0), stop=(k == CHUNKS - 1),
                    )

                if PAIR_EMIT:
                    for j in range(CHUNKS // 2):
                        ka, kb = 2 * j, 2 * j + 1
                        p1a = emit_L1(ka)
                        h1a = emit_evac1(ka, p1a)
                        p1b = emit_L1(kb)
                        h1b = emit_evac1(kb, p1b)
                        p2a = emit_L2(h1a)
                        p2b = emit_L2(h1b)
                        h2a = emit_evac2(ka, p2a)
                        h2b = emit_evac2(kb, p2b)
                        emit_L3(ka, h2a)
                        emit_L3(kb, h2b)
                else:
                    for k in range(CHUNKS):
                        if CHAIN_GP and k == 10:
                            th = tmp.tile([128, FREE], f32, tag="th", name="th")
                            nc.scalar.activation(th[:], y_pl[:], Act.Tanh)
                            t1 = tmp.tile([128, FREE], f32, tag="t1", name="t1")
                            nc.gpsimd.tensor_scalar_mul(t1[:], th[:], a_coef)
                            nc.gpsimd.tensor_scalar_add(t1[:], t1[:], b_coef)
                            t2 = tmp.tile([128, FREE], f32, tag="t2", name="t2")
                            nc.gpsimd.tensor_tensor(t2[:], t1[:], dw_t[:], Alu.mult)
                            t3 = tmp.tile([128, FREE], f32, tag="t3", name="t3")
                            nc.gpsimd.tensor_scalar_mul(t3[:], y_pl[:], 1.0 - DT)
                            nc.gpsimd.tensor_tensor(t3[:], t3[:], t2[:], Alu.add)
                        if T3_EARLY:
                            if k == 10:
                                th = tmp.tile([128, FREE], f32, tag="th", name="th")
                                nc.scalar.activation(th[:], y_pl[:], Act.Tanh)
                            elif k == 14:
                                t1 = tmp.tile([128, FREE], f32, tag="t1", name="t1")
                                nc.vector.tensor_scalar(t1[:], th[:], a_coef, b_coef,
                                                        Alu.mult, Alu.add)
                            elif k == 18:
                                t2 = tmp.tile([128, FREE], f32, tag="t2", name="t2")
                                nc.gpsimd.tensor_tensor(t2[:], t1[:], dw_t[:], Alu.mult)
                            elif k == 24:
                                t3 = tmp.tile([128, FREE], f32, tag="t3", name="t3")
                                nc.vector.scalar_tensor_tensor(t3[:], y_pl[:], 1.0 - DT,
                                                               t2[:], Alu.mult, Alu.add)
                        p1 = emit_L1(k)
                        h1 = emit_evac1(k, p1)
                        p2 = emit_L2(h1)
                        h2 = emit_evac2(k, p2)
                        emit_L3(k, h2)

                if SPLIT_TAIL:
                    # q rows evacuate first (ACT) and alone feed the K=32
                    # broadcast; z rows evacuate on DVE off the critical path
                    oq = tmp.tile([32, FREE], f32r, tag="oq", name="oq")
                    nc.vector.tensor_scalar(oq[:], p3[0:32, :], B3[0:32, 0:1], None, Alu.add)
                    pqpool, pqtag = (ps3, "p3") if PS2_4 else (psq, "pq")
                    pq = pqpool.tile([128, FREE], f32, tag=pqtag, name="pq")
                    nc.tensor.matmul(pq[:], BB[0:32, :], oq[:])
                    nc.vector.tensor_scalar(out3s[32:128, :], p3[32:128, :],
                                            B3[32:128, 0:1], None, Alu.add)
                else:
                    # out3s = psum3 + per-partition bias
                    if TAIL_ACT:
                        nc.scalar.activation(out3s[:], p3[:], Act.Identity, bias=B3[:, 0:1])
                    else:
                        nc.vector.tensor_scalar(out3s[:], p3[:], B3[:, 0:1], None, Alu.add)

                    # qrep = broadcast dt*q to component quarters (via PE)
                    pqpool, pqtag = (ps3, "p3") if PS2_4 else (psq, "pq")
                    pq = pqpool.tile([128, FREE], f32, tag=pqtag, name="pq")
                    nc.tensor.matmul(pq[:], BB[:], out3s[:])

                # ---- elementwise state update ----
                if not T3_EARLY and not CHAIN_GP:
                    th = tmp.tile([128, FREE], f32, tag="th", name="th")
                    nc.scalar.activation(th[:], y_pl[:], Act.Tanh)
                    t1 = tmp.tile([128, FREE], f32, tag="t1", name="t1")
                    nc.vector.tensor_scalar(t1[:], th[:], a_coef, b_coef, Alu.mult, Alu.add)
                    t2 = tmp.tile([128, FREE], f32, tag="t2", name="t2")
                    nc.vector.scalar_tensor_tensor(t2[:], t1[:], 1.0, dw_t[:], Alu.mult, Alu.mult)
                    t3 = tmp.tile([128, FREE], f32, tag="t3", name="t3")
                    nc.vector.scalar_tensor_tensor(t3[:], y_pl[:], 1.0 - DT, t2[:], Alu.mult, Alu.add)
                nc.vector.scalar_tensor_tensor(y_pl[:], t3[:], 0.0, pq[:], Alu.add, Alu.add)

                sqA = tmp.tile([32, FREE], f32, tag="sqA", name="sqA")
                p6 = tmp.tile([128, FREE], f32, tag="p6", name="p6")
                if SPLIT_TAIL:
                    nc.gpsimd.tensor_tensor(sqA[:], oq[:], oq[:], Alu.mult)
                    nc.gpsimd.tensor_tensor(accA[:], accA[:], sqA[:], Alu.add)
                    nc.gpsimd.tensor_tensor(p6[32:128, :], out3s[32:128, :],
                                            dw_t[32:128, :], Alu.mult)
                    nc.gpsimd.tensor_tensor(accP[32:128, :], accP[32:128, :],
                                            p6[32:128, :], Alu.add)
                elif GP_ACC:
                    nc.gpsimd.tensor_tensor(sqA[:], out3s[0:32, :], out3s[0:32, :], Alu.mult)
                    nc.gpsimd.tensor_tensor(accA[:], accA[:], sqA[:], Alu.add)
                    nc.gpsimd.tensor_tensor(p6[:], out3s[:], dw_t[:], Alu.mult)
                    nc.gpsimd.tensor_tensor(accP[:], accP[:], p6[:], Alu.add)
                else:
                    nc.scalar.activation(sqA[:], out3s[0:32, :], Act.Square)
                    nc.vector.scalar_tensor_tensor(accA[:], accA[:], 0.0, sqA[:], Alu.add, Alu.add)
                    nc.vector.scalar_tensor_tensor(p6[:], out3s[:], 1.0, dw_t[:], Alu.mult, Alu.mult)
                    nc.vector.scalar_tensor_tensor(accP[:], accP[:], 0.0, p6[:], Alu.add, Alu.add)

            # ---- final loss assembly ----
            ysq = tmp.tile([128, FREE], f32r, tag="ysq", name="ysq")
            nc.scalar.activation(ysq[:], y_pl[:], Act.Square)
            p_term = ps1.tile([32, FREE], f32, tag="p1", name="pterm")
            nc.tensor.matmul(p_term[:], TT[:], ysq[:])
            accPr = tmp.tile([128, FREE], f32r, tag="accPr", name="accPr")
            nc.vector.tensor_copy(accPr[:], accP[:])
            p_P = ps2.tile([32, FREE], f32, tag="p2", name="pP")
            nc.tensor.matmul(p_P[:], TT[:], accPr[:])

            Pg = tmp.tile([32, FREE], f32, tag="Pg", name="Pg")
            nc.vector.tensor_scalar(Pg[:], p_P[:], 0.0, None, Alu.add)
            Tg = tmp.tile([32, FREE], f32, tag="Tg", name="Tg")
            nc.vector.tensor_scalar(Tg[:], p_term[:], 0.0, None, Alu.add)
            D1 = tmp.tile([32, FREE], f32, tag="D1", name="D1")
            nc.vector.scalar_tensor_tensor(D1[:], accA[:], -0.5 / DT, Pg[:], Alu.mult, Alu.add)
            D2 = tmp.tile([32, FREE], f32, tag="D2", name="D2")
            nc.vector.scalar_tensor_tensor(D2[:], Tg[:], -1.0, D1[:], Alu.mult, Alu.add)
            # add Y0 (runtime input, broadcast from SCAL[0])
            sc = consts.tile([4, 1], f32, tag="SCAL", name="SCAL")
            nc.gpsimd.dma_start(sc[:], d_SC)
            y0b = consts.tile([32, 1], f32, tag="y0b", name="y0b")
            nc.gpsimd.dma_start(y0b[:], bass.AP(tensor=d_SC.tensor, offset=0, ap=[[0, 32], [1, 1]]))
            D3 = tmp.tile([32, FREE], f32, tag="D3", name="D3")
            nc.vector.tensor_scalar(D3[:], D2[:], y0b[:, 0:1], None, Alu.add)

            dsq = tmp.tile([32, FREE], f32, tag="dsq", name="dsq")
            res = state.tile([32, 1], f32, tag="res", name="res")
            nc.scalar.activation(dsq[:], D3[:], Act.Square, accum_out=res[:])
            nc.sync.dma_start(d_res, res[:])

    return nc


LAST_EXEC_NS = None
LAST_TRACE_DIR = None


def kernel(**inputs) -> np.ndarray:
    global LAST_EXEC_NS, LAST_TRACE_DIR
    from concourse.bass_utils import run_bass_kernel_spmd
    _install_wait_splitter()
    _install_ldw_opt()

    host = prep_host(inputs)

    nc = build_program()

    scal = np.array([[host["Y0"]], [-0.5 / DT], [0.1 * SQRT_DT], [0.2 * SQRT_DT]], np.float32)
    shared = dict(
        W1S=host["W1S"], CB=host["CB"], W2=host["W2"], B2=host["B2"],
        W3S=host["W3S"], B3=host["B3"], BB=host["BB"], TT=host["TT"],
        YINIT=host["YINIT"], SCAL=scal,
    )
    in_maps = []
    for r in range(N_CORES):
        m = dict(shared)
        m["dWt"] = host["dWt_cores"][r]
        in_maps.append(m)

    trace = os.environ.get("BSDE_TRACE", "0") == "1"
    kw = {}
    if trace:
        kw["trace"] = True
        kw["tmpdir"] = os.environ.get("BSDE_TRACE_DIR") or None
    out = run_bass_kernel_spmd(nc, in_maps, list(range(N_CORES)), **kw)
    LAST_EXEC_NS = getattr(out, "exec_time_ns", None)
    total = np.float64(0.0)
    for r in range(N_CORES):
        total += np.sum(out.results[r]["res"].astype(np.float64))
    return np.float32(total / BATCH)


def _build_in_maps(host):
    scal = np.array([[host["Y0"]], [-0.5 / DT], [0.1 * SQRT_DT], [0.2 * SQRT_DT]], np.float32)
    shared = dict(
        W1S=host["W1S"], CB=host["CB"], W2=host["W2"], B2=host["B2"],
        W3S=host["W3S"], B3=host["B3"], BB=host["BB"], TT=host["TT"],
        YINIT=host["YINIT"], SCAL=scal,
    )
    in_maps = []
    for r in range(N_CORES):
        m = dict(shared)
        m["dWt"] = host["dWt_cores"][r]
        in_maps.append(m)
    return in_maps


def timed_run(nc, in_maps, iters=7):
    """Mirror bass2jax.run_bass_via_pjrt's multi-core path, but keep inputs
    device-resident and time steady-state executions. Returns (results_core0,
    sorted wall times in ns per call)."""
    import time
    import jax
    from jax.sharding import Mesh, PartitionSpec, NamedSharding
    from jax.experimental.shard_map import shard_map
    from concourse import bass2jax, mybir

    bass2jax.install_neuronx_cc_hook()
    n_cores = N_CORES

    in_names, out_names, out_avals, zero_outs = [], [], [], []
    for alloc in nc.m.functions[0].allocations:
        if not isinstance(alloc, mybir.MemoryLocationSet):
            continue
        name = alloc.memorylocations[0].name
        if alloc.kind == "ExternalInput":
            in_names.append(name)
        elif alloc.kind == "ExternalOutput":
            out_names.append(name)
            shape = tuple(alloc.tensor_shape)
            dtype = mybir.dt.np(alloc.dtype)
            out_avals.append(jax.core.ShapedArray(shape, dtype))
            zero_outs.append(np.zeros(shape, dtype))
    n_params = len(in_names)
    n_outs = len(out_avals)
    all_names = in_names + out_names
    donate = tuple(range(n_params, n_params + n_outs))

    def _body(*args):
        outs = bass2jax._bass_exec_p.bind(
            *list(args),
            out_avals=tuple(out_avals),
            in_names=tuple(all_names),
            out_names=tuple(out_names),
            lowering_input_output_aliases=(),
            sim_require_finite=True,
            sim_require_nnan=True,
            nc=nc,
        )
        return tuple(outs)

    devices = jax.devices()[:n_cores]
    mesh = Mesh(np.asarray(devices), ("core",))
    in_specs = (PartitionSpec("core"),) * (n_params + n_outs)
    out_specs = (PartitionSpec("core"),) * len(out_names)
    sharded = jax.jit(
        shard_map(_body, mesh=mesh, in_specs=in_specs, out_specs=out_specs, check_rep=False),
        donate_argnums=donate,
        keep_unused=True,
    )
    concat_in = [
        np.concatenate([np.asarray(in_maps[c][nm]) for c in range(n_cores)], axis=0)
        for nm in in_names
    ]
    sh = NamedSharding(mesh, PartitionSpec("core"))
    dev_in = [jax.device_put(a, sh) for a in concat_in]
    concat_zeros = [np.zeros((n_cores * z.shape[0], *z.shape[1:]), z.dtype) for z in zero_outs]

    out = sharded(*dev_in, *concat_zeros)   # warm-up / compile
    jax.block_until_ready(out)
    times = []
    for _ in range(iters):
        zz = [np.zeros((n_cores * z.shape[0], *z.shape[1:]), z.dtype) for z in zero_outs]
        t0 = time.perf_counter_ns()
        out = sharded(*dev_in, *zz)
        jax.block_until_ready(out)
        times.append(time.perf_counter_ns() - t0)
    res0 = {
        nm: np.asarray(out[i]).reshape(n_cores, *out_avals[i].shape)
        for i, nm in enumerate(out_names)
    }
    return res0, sorted(times)


if __name__ == "__main__":
    rng = np.random.default_rng(0)
    fake = {
        "y0": rng.standard_normal(3).astype(np.float32),
        "Y0": np.zeros((1, 1), np.float32),
        "qW1": rng.standard_normal((4, 64)).astype(np.float32) * 0.5,
        "qb1": np.zeros(64, np.float32),
        "qW2": rng.standard_normal((64, 64)).astype(np.float32) * 0.12,
        "qb2": np.zeros(64, np.float32),
        "qW3": rng.standard_normal((64, 1)).astype(np.float32) * 0.12,
        "qb3": np.zeros(1, np.float32),
        "zW1": rng.standard_normal((4, 64)).astype(np.float32) * 0.5,
        "zb1": np.zeros(64, np.float32),
        "zW2": rng.standard_normal((64, 64)).astype(np.float32) * 0.12,
        "zb2": np.zeros(64, np.float32),
        "zW3": rng.standard_normal((64, 3)).astype(np.float32) * 0.12,
        "zb3": np.zeros(3, np.float32),
        "dW": rng.standard_normal((N_STEPS, BATCH, 3)).astype(np.float32),
    }
    print(kernel(**fake))



# revision 38
# speedup vs baseline: 1.2529x; 1.0010x over previous
"""DeepBSDE forward-loss kernel for Trainium2 (8 NeuronCores, data-parallel).

Math (per sample b, 50 steps, dt=0.02):
    x_n = [t_n, y_n]                       (4 features)
    z_n = MLP_z(x_n)   (4->64->64->3, relu)
    q_n = MLP_q(x_n)   (4->64->64->1, relu)
    y_{n+1} = (1-dt) y_n + dt q_n + (0.2 + 0.1 tanh(y_n)) * sqrt(dt) * dW_n
    Y_final = Y0 - 0.5 dt sum_n q_n^2 + sum_n z_n . (sqrt(dt) dW_n)
    out = mean_b (Y_final - |y_final|^2)^2

Device layout (per core, B_loc = 16384 = 32 chunks x 512):
    every per-sample state lives in a [128, 512] SBUF tile:
      partition k        (k in 0..31)   : q-slot of chunk k
      partition 32+32i+k (i in 0..2)    : vector component i of chunk k
      free c                            : sample index b = k*512 + c
    The two MLPs are fused: hidden = [q-hidden(64) ; z-hidden(64)] = 128.
    Per step the PE streams: L1 (K=3), L2 (K=128), L3 (per-chunk sparse
    [128,128] stationaries accumulated into ONE packed psum bank), plus one
    broadcast matmul that replicates dt*q into the 3 component quarters.
"""

import sys
import os

for _p in ("/opt/trn_rl_repo", "/root/.axon_site/_ro/trn_rl_repo"):
    if os.path.isdir(_p) and _p not in sys.path:
        sys.path.insert(0, _p)

import numpy as np

DT = 0.02
SQRT_DT = float(np.sqrt(np.float32(DT)))
N_STEPS = 50
BATCH = 131072
DIM = 3
N_CORES = 8
B_LOC = BATCH // N_CORES          # 16384
CHUNKS = 32
FREE = B_LOC // CHUNKS            # 512

# dtype knobs.
#  - L1/BB/TT matmuls read fp32 state; run them as float32r (same 4-byte
#    storage, 1 cycle/row on the PE at moving size >= 256 vs 4 for fp32).
#  - hidden activations h1/h2 and the L2/L3 weights run in bf16: same PE
#    rate as f32r but half the ACT/DVE evacuation cost and half the
#    weight-load traffic.
MM_HID_F32 = os.environ.get("BSDE_HID_F32", "0") == "1"

# how many h2 evacuations run on the scalar (ACT) engine instead of DVE,
# to balance the two engines' per-step load.
ACT_H2 = int(os.environ.get("BSDE_ACT_H2", "2"))

# offload the final-reduction accumulation ops (sqA product, accA/accP adds,
# p6 product) to the otherwise-idle GPSIMD engine.
GP_ACC = os.environ.get("BSDE_GP_ACC", "1") == "1"

# L1 matmuls in bf16 (stationary + a per-step bf16 copy of y): halves the
# f32r weight-load time on the PE at a tiny precision cost.
L1_BF16 = os.environ.get("BSDE_L1_BF16", "0") == "1"

# split the out3s evacuation: q rows via ACT feed a K=32 broadcast matmul so
# the z-row evacuation leaves the step-boundary critical path.
SPLIT_TAIL = os.environ.get("BSDE_SPLIT_TAIL", "0") == "1"

# emit PE work pair-adjacent (L1,L1,L2,L2,L3,L3 per chunk pair) so the two
# W2 loads sit back-to-back for weight-load pull-ahead.
PAIR_EMIT = os.environ.get("BSDE_PAIR_EMIT", "0") == "1"

# deeper activation rings to absorb evacuation jitter (SBUF has headroom)
RING_H = os.environ.get("BSDE_RING_H", "1") == "1"

# give ps2 a 4th bank by allocating pq from ps3's ring (p3's bank is free by
# the time the broadcast runs; the WAR dep is exactly the required ordering)
PS2_4 = os.environ.get("BSDE_PS2_4", "1") == "1"

# evacuate out3s on the ACT engine (Identity + vector bias): at the step tail
# ACT's queue is empty while DVE is still draining h2 evacuations, so the
# p3 -> out3s -> broadcast -> y-add chain starts sooner.
TAIL_ACT = os.environ.get("BSDE_TAIL_ACT", "1") == "1"

# emit the diffusion chain (tanh, t1, t2, t3) spread through the chunk loop
# so only the final y-add remains in the step tail's DVE queue.
T3_EARLY = os.environ.get("BSDE_T3_EARLY", "0") == "1"

# run the diffusion chain (t1,t2,t3) on the idle GPSIMD queue (split into
# single-ALU ops), with tanh emitted mid-loop on ACT: the chain then finishes
# mid-step instead of serializing at the end of DVE's FIFO, so the step tail
# is just broadcast + y-add.
CHAIN_GP = os.environ.get("BSDE_CHAIN_GP", "0") == "1"

# assign the ACT-side h2 evacuations to the LAST chunks instead of the first:
# ACT helps drain the end of the h2 stream right before the step tail.
ACT_H2_LATE = os.environ.get("BSDE_ACT_H2_LATE", "1") == "1"

# number of trailing h1 evacuations moved from ACT to DVE so ACT reaches the
# out3s evacuation (start of the step-tail chain) sooner.
DVE_H1 = int(os.environ.get("BSDE_DVE_H1", "0"))

# number of LEADING h1 evacuations moved from ACT to DVE: at step start DVE
# is idle (no h2 backlog yet), so this sheds ACT load without delaying the PE.
DVE_H1_EARLY = int(os.environ.get("BSDE_DVE_H1_EARLY", "2"))

# number of LEADING h2 evacuations moved from DVE to ACT (ACT has slack at
# step start now that the first h1s run on DVE).
ACT_H2_EARLY = int(os.environ.get("BSDE_ACT_H2_EARLY", "0"))

# run t1 = a*tanh + b on ACT (Copy with scalar scale+bias) instead of DVE.
T1_ACT = os.environ.get("BSDE_T1_ACT", "0") == "1"

# shift the late ACT h2 pair one chunk earlier ({29,30} instead of {30,31}),
# leaving h2[31] on the mostly-drained DVE so L3[31] is not gated by ACT.
ACT_H2_SHIFT = os.environ.get("BSDE_ACT_H2_SHIFT", "0") == "1"

# issue the per-step dW load on the sync HWDGE queue instead of the GPSIMD
# software-DMA queue (stops interleaving with the accumulation ops).
DW_SYNC = os.environ.get("BSDE_DW_SYNC", "0") == "1"

# If set (by the timing harness), the device loop runs this many steps while
# all I/O shapes stay identical — lets wall-clock differencing isolate the
# per-step device time from RPC/transfer overhead.
LOOP_STEPS = None


def _np_f32(x):
    return np.ascontiguousarray(np.asarray(x, dtype=np.float32))


def prep_host(inputs):
    """Build all device-side arrays (numpy fp32) from the raw problem inputs."""
    i = {k: _np_f32(v) for k, v in inputs.items()}
    qW1, qb1 = i["qW1"], i["qb1"]
    qW2, qb2 = i["qW2"], i["qb2"]
    qW3, qb3 = i["qW3"], i["qb3"]
    zW1, zb1 = i["zW1"], i["zb1"]
    zW2, zb2 = i["zW2"], i["zb2"]
    zW3, zb3 = i["zW3"], i["zb3"]
    y0 = i["y0"]
    Y0 = float(i["Y0"].reshape(-1)[0])
    dW = i["dW"]

    W1cat = np.concatenate([qW1, zW1], axis=1)          # [4, 128]
    b1cat = np.concatenate([qb1, zb1])                  # [128]

    # L1 reads the packed y state directly: per-chunk sparse stationaries.
    # chunk k's component i lives at partition 32 + 32*i + k.
    W1S = np.zeros((CHUNKS, 128, 128), np.float32)
    for k in range(CHUNKS):
        for comp in range(3):
            W1S[k, 32 + 32 * comp + k, :] = W1cat[1 + comp, :]
    W1S = np.ascontiguousarray(W1S.transpose(1, 0, 2).reshape(128, CHUNKS * 128))

    # per-step bias for the L1 relu evacuation: c_n = t_n * W1cat[0] + b1cat
    ts = (np.arange(N_STEPS, dtype=np.float32) * np.float32(DT))
    CB = (ts[None, :] * W1cat[0][:, None] + b1cat[:, None]).astype(np.float32)  # [128, 50]

    W2 = np.zeros((128, 128), np.float32)
    W2[0:64, 0:64] = qW2
    W2[64:128, 64:128] = zW2
    B2 = b1cat * 0.0
    B2 = np.concatenate([qb2, zb2]).astype(np.float32).reshape(128, 1)

    # L3: per-chunk sparse stationaries [128, 32*128]
    W3S = np.zeros((CHUNKS, 128, 128), np.float32)
    for k in range(CHUNKS):
        W3S[k, 0:64, k] = DT * qW3[:, 0]
        for comp in range(3):
            W3S[k, 64:128, 32 + 32 * comp + k] = SQRT_DT * zW3[:, comp]
    W3S = np.ascontiguousarray(W3S.transpose(1, 0, 2).reshape(128, CHUNKS * 128))

    B3 = np.zeros((128, 1), np.float32)
    B3[0:32, 0] = DT * qb3[0]
    for comp in range(3):
        B3[32 + 32 * comp:64 + 32 * comp, 0] = SQRT_DT * zb3[comp]

    # broadcast matmul: qrep[32+32i+k] = out3s[k]
    BB = np.zeros((128, 128), np.float32)
    for k in range(CHUNKS):
        for comp in range(3):
            BB[k, 32 + 32 * comp + k] = 1.0

    # final reduction: col j sums the 3 components of chunk j
    TT = np.zeros((128, 32), np.float32)
    for j in range(CHUNKS):
        for comp in range(3):
            TT[32 + 32 * comp + j, j] = 1.0

    # initial y state, broadcast to full [128, 512] (q-slot rows zero)
    YINIT = np.zeros((128, FREE), np.float32)
    for comp in range(3):
        YINIT[32 + 32 * comp:64 + 32 * comp, :] = y0[comp]

    # per-core dW, transposed to [steps, comp, b_loc]; row block 0 (the q-slot
    # partitions) is zeros so a single full-tile DMA initializes everything
    dWt_cores = []
    for r in range(N_CORES):
        sl = dW[:N_STEPS, r * B_LOC:(r + 1) * B_LOC, :]     # [N_STEPS, B_loc, 3]
        t = np.zeros((N_STEPS, 4 * B_LOC), np.float32)
        t[:, B_LOC:] = sl.transpose(0, 2, 1).reshape(N_STEPS, 3 * B_LOC)
        dWt_cores.append(t)

    return dict(
        W1S=W1S, CB=CB, W2=W2, B2=B2, W3S=W3S, B3=B3, BB=BB, TT=TT,
        YINIT=YINIT, dWt_cores=dWt_cores, Y0=Y0,
    )


def _split_sync_waits(bir: dict) -> dict:
    """Walrus in this toolchain accepts only ~1 sync wait per instruction.
    Hoist extra waits onto standalone EventSemaphore instructions inserted
    just before, on the same engine (waits-only, so semantics unchanged)."""
    n = 0
    for fn in bir.get("functions", []):
        for bb in fn.get("blocks", []):
            out = []
            for ins in bb.get("instructions", []):
                si = ins.get("sync_info")
                waits = (si or {}).get("on_wait") or []
                if len(waits) > 1:
                    for w in waits[:-1]:
                        n += 1
                        out.append({
                            "engine": ins["engine"],
                            "ins": [],
                            "outs": [],
                            "name": f"bsdewait{n}_{ins['name']}",
                            "opcode": "EventSemaphore",
                            "debug": ins.get("debug", 0),
                            "sync_info": {"on_update": [], "on_wait": [w]},
                        })
                    si["on_wait"] = [waits[-1]]
                out.append(ins)
            bb["instructions"] = out
    return bir


def _install_ldw_opt():
    """walrus is invoked with --enable-ldw-opt=false; flip it on so repeated/
    adjacent stationary loads are optimized (gated by BSDE_LDW_OPT)."""
    from concourse import bass_utils
    if os.environ.get("BSDE_LDW_OPT", "0") != "1":
        return
    if getattr(bass_utils, "_bsde_ldwopt_installed", False):
        return
    orig = bass_utils.run_command

    def wrapped(cmd, **kw):
        if isinstance(cmd, list):
            cmd = ["--enable-ldw-opt=true" if c == "--enable-ldw-opt=false" else c
                   for c in cmd]
        return orig(cmd, **kw)

    bass_utils.run_command = wrapped
    bass_utils._bsde_ldwopt_installed = True


def _install_wait_splitter():
    import json as _json
    from concourse import bass2jax, bass_utils
    if getattr(bass_utils, "_bsde_split_installed", False):
        return
    orig = bass_utils.compile_bir_kernel

    def wrapped(bir_json, tmpdir, neff_name="file.neff"):
        bir = _json.loads(bir_json)
        _split_sync_waits(bir)
        return orig(_json.dumps(bir).encode(), tmpdir, neff_name)

    bass_utils.compile_bir_kernel = wrapped
    bass2jax.compile_bir_kernel = wrapped
    bass_utils._bsde_split_installed = True


def build_program():
    """Build the Bass program (same for all cores). Returns (nc, meta)."""
    from concourse import bass, mybir, tile

    f32 = mybir.dt.float32
    f32r = mybir.dt.float32r
    hdt = f32 if MM_HID_F32 else mybir.dt.bfloat16
    Alu = mybir.AluOpType
    Act = mybir.ActivationFunctionType

    def R(ap):
        # reinterpret fp32 data as float32r for full-rate PE streaming
        return ap.bitcast(f32r)

    def H(ap):
        # hidden-path operand: native bf16, or f32->f32r bitcast in fallback
        return R(ap) if hdt == f32 else ap

    nc = bass.Bass("TRN2", target_bir_lowering=False, debug=False)

    # --- dram I/O ---
    d_dWt = nc.dram_tensor("dWt", [N_STEPS, 4 * B_LOC], f32, kind="ExternalInput").ap()
    d_W1S = nc.dram_tensor("W1S", [128, CHUNKS * 128], f32r, kind="ExternalInput").ap()
    d_CB = nc.dram_tensor("CB", [128, N_STEPS], f32, kind="ExternalInput").ap()
    d_W2 = nc.dram_tensor("W2", [128, 128], f32, kind="ExternalInput").ap()
    d_B2 = nc.dram_tensor("B2", [128, 1], f32, kind="ExternalInput").ap()
    d_W3S = nc.dram_tensor("W3S", [128, CHUNKS * 128], f32, kind="ExternalInput").ap()
    d_B3 = nc.dram_tensor("B3", [128, 1], f32, kind="ExternalInput").ap()
    d_BB = nc.dram_tensor("BB", [128, 128], f32r, kind="ExternalInput").ap()
    d_TT = nc.dram_tensor("TT", [128, 32], f32r, kind="ExternalInput").ap()
    d_YI = nc.dram_tensor("YINIT", [128, FREE], f32r, kind="ExternalInput").ap()
    d_SC = nc.dram_tensor("SCAL", [4, 1], f32, kind="ExternalInput").ap()  # [Y0; -0.5/dt; a; b]
    d_res = nc.dram_tensor("res", [32, 1], f32, kind="ExternalOutput").ap()

    a_coef = 0.1 * SQRT_DT
    b_coef = 0.2 * SQRT_DT

    with tile.TileContext(nc) as tc:
        with (
            tc.tile_pool(name="consts", bufs=1) as consts,
            tc.tile_pool(name="state", bufs=1) as state,
            tc.tile_pool(name="h1p", bufs=(6 if RING_H else 3)) as h1pool,
            tc.tile_pool(name="h2p", bufs=(8 if RING_H else 4)) as h2pool,
            tc.tile_pool(name="tmp", bufs=1) as tmp,
            tc.tile_pool(name="dwp", bufs=8) as dwp,
            tc.tile_pool(name="ps1", bufs=3, space="PSUM") as ps1,
            tc.tile_pool(name="ps2", bufs=(4 if PS2_4 else 3), space="PSUM") as ps2,
            tc.tile_pool(name="ps3", bufs=1, space="PSUM") as ps3,
            tc.tile_pool(name="psq", bufs=1, space="PSUM") as psq,
        ):
            # ---- load constants into SBUF ----
            _dma_eng = [nc.sync, nc.scalar, nc.gpsimd]
            _dma_i = [0]

            def load_const(name, dram_ap, shape, dt_):
                t = consts.tile(shape, dt_, tag=name)
                eng = _dma_eng[_dma_i[0] % len(_dma_eng)]
                _dma_i[0] += 1
                eng.dma_start(t[:], dram_ap)
                return t

            W1S = load_const("W1S", d_W1S, [128, CHUNKS * 128], f32r)
            CB = load_const("CB", d_CB, [128, N_STEPS], f32)
            W2 = load_const("W2", d_W2, [128, 128], f32)
            B2 = load_const("B2", d_B2, [128, 1], f32)
            W3S = load_const("W3S", d_W3S, [128, CHUNKS * 128], f32)
            B3 = load_const("B3", d_B3, [128, 1], f32)
            BB = load_const("BB", d_BB, [128, 128], f32r)
            TT = load_const("TT", d_TT, [128, 32], f32r)

            if L1_BF16:
                W1Sb = consts.tile([128, CHUNKS * 128], mybir.dt.bfloat16,
                                   tag="W1Sb", name="W1Sb")
                nc.vector.tensor_copy(W1Sb[:], W1S[:])

            if hdt == f32:
                W2m, W3Sm = W2, W3S
            else:
                W2m = consts.tile([128, 128], hdt, tag="W2m", name="W2m")
                nc.vector.tensor_copy(W2m[:], W2[:])
                W3Sm = consts.tile([128, CHUNKS * 128], hdt, tag="W3Sm", name="W3Sm")
                nc.vector.tensor_copy(W3Sm[:], W3S[:])

            # ---- persistent state ----
            y_pl = state.tile([128, FREE], f32r, tag="y_pl", name="y_pl")
            nc.gpsimd.dma_start(y_pl[:], d_YI)
            accA = state.tile([32, FREE], f32, tag="accA", name="accA")
            nc.vector.memset(accA[:], 0.0)
            accP = state.tile([128, FREE], f32, tag="accP", name="accP")
            nc.vector.memset(accP[:], 0.0)
            out3s = state.tile([128, FREE], f32r, tag="out3s", name="out3s")
            if L1_BF16:
                y_plb = state.tile([128, FREE], mybir.dt.bfloat16,
                                   tag="y_plb", name="y_plb")

            # ---- time loop ----
            # Chunk-interleaved pipeline: per chunk L1 -> (ACT evac) -> L2 ->
            # (DVE/ACT evac) -> L3-accumulate; the psum rings provide the
            # cross-chunk overlap. SBUF-only elementwise work runs on the
            # otherwise-idle GPSIMD.
            n_loop = N_STEPS if LOOP_STEPS is None else LOOP_STEPS
            for n in range(n_loop):
                dw_t = dwp.tile([128, FREE], f32, tag="dw", name="dw")
                (nc.sync if DW_SYNC else nc.gpsimd).dma_start(dw_t[:, :], d_dWt[n, :])

                p3 = ps3.tile([128, FREE], f32, tag="p3", name="p3")
                cb_n = CB[:, n:n + 1]

                if L1_BF16:
                    nc.gpsimd.tensor_copy(y_plb[:], y_pl[:])
                    l1_w, l1_y = W1Sb, y_plb
                else:
                    l1_w, l1_y = W1S, y_pl

                def emit_L1(k):
                    p1 = ps1.tile([128, FREE], f32, tag="p1", name="p1")
                    nc.tensor.matmul(p1[:], l1_w[:, k * 128:(k + 1) * 128], l1_y[:])
                    return p1

                def emit_evac1(k, p1):
                    h1 = h1pool.tile([128, FREE], hdt, tag="h1", name="h1")
                    if (DVE_H1 and k >= CHUNKS - DVE_H1) or (DVE_H1_EARLY and k < DVE_H1_EARLY):
                        nc.vector.tensor_scalar(h1[:], p1[:], cb_n, 0.0,
                                                Alu.add, Alu.max)
                    else:
                        nc.scalar.activation(h1[:], p1[:], Act.Relu, bias=cb_n)
                    return h1

                def emit_L2(h1):
                    p2 = ps2.tile([128, FREE], f32, tag="p2", name="p2")
                    nc.tensor.matmul(p2[:], H(W2m[:]), H(h1[:]))
                    return p2

                def emit_evac2(k, p2):
                    h2 = h2pool.tile([128, FREE], hdt, tag="h2", name="h2")
                    if ACT_H2_LATE:
                        hi = CHUNKS - 1 if ACT_H2_SHIFT else CHUNKS
                        on_act = (hi - ACT_H2) <= k < hi
                    else:
                        on_act = k < ACT_H2
                    if on_act or (ACT_H2_EARLY and k < ACT_H2_EARLY):
                        nc.scalar.activation(h2[:], p2[:], Act.Relu, bias=B2[:, 0:1])
                    else:
                        nc.vector.tensor_scalar(h2[:], p2[:], B2[:, 0:1], 0.0,
                                                Alu.add, Alu.max)
                    return h2

                def emit_L3(k, h2):
                    nc.tensor.matmul(
                        p3[:], H(W3Sm[:, k * 128:(k + 1) * 128]), H(h2[:]),
                        start=(k == 0), stop=(k == CHUNKS - 1),
                    )

                if PAIR_EMIT:
                    for j in range(CHUNKS // 2):
                        ka, kb = 2 * j, 2 * j + 1
                        p1a = emit_L1(ka)
                        h1a = emit_evac1(ka, p1a)
                        p1b = emit_L1(kb)
                        h1b = emit_evac1(kb, p1b)
                        p2a = emit_L2(h1a)
                        p2b = emit_L2(h1b)
                        h2a = emit_evac2(ka, p2a)
                        h2b = emit_evac2(kb, p2b)
                        emit_L3(ka, h2a)
                        emit_L3(kb, h2b)
                else:
                    for k in range(CHUNKS):
                        if CHAIN_GP and k == 10:
                            th = tmp.tile([128, FREE], f32, tag="th", name="th")
                            nc.scalar.activation(th[:], y_pl[:], Act.Tanh)
                            t1 = tmp.tile([128, FREE], f32, tag="t1", name="t1")
                            nc.gpsimd.tensor_scalar_mul(t1[:], th[:], a_coef)
                            nc.gpsimd.tensor_scalar_add(t1[:], t1[:], b_coef)
                            t2 = tmp.tile([128, FREE], f32, tag="t2", name="t2")
                            nc.gpsimd.tensor_tensor(t2[:], t1[:], dw_t[:], Alu.mult)
                            t3 = tmp.tile([128, FREE], f32, tag="t3", name="t3")
                            nc.gpsimd.tensor_scalar_mul(t3[:], y_pl[:], 1.0 - DT)
                            nc.gpsimd.tensor_tensor(t3[:], t3[:], t2[:], Alu.add)
                        if T3_EARLY:
                            if k == 10:
                                th = tmp.tile([128, FREE], f32, tag="th", name="th")
                                nc.scalar.activation(th[:], y_pl[:], Act.Tanh)
                            elif k == 14:
                                t1 = tmp.tile([128, FREE], f32, tag="t1", name="t1")
                                nc.vector.tensor_scalar(t1[:], th[:], a_coef, b_coef,
                                                        Alu.mult, Alu.add)
                            elif k == 18:
                                t2 = tmp.tile([128, FREE], f32, tag="t2", name="t2")
                                nc.gpsimd.tensor_tensor(t2[:], t1[:], dw_t[:], Alu.mult)
                            elif k == 24:
                                t3 = tmp.tile([128, FREE], f32, tag="t3", name="t3")
                                nc.vector.scalar_tensor_tensor(t3[:], y_pl[:], 1.0 - DT,
                                                               t2[:], Alu.mult, Alu.add)
                        p1 = emit_L1(k)
                        h1 = emit_evac1(k, p1)
                        p2 = emit_L2(h1)
                        h2 = emit_evac2(k, p2)
                        emit_L3(k, h2)

                if SPLIT_TAIL:
                    # q rows evacuate first (ACT) and alone feed the K=32
                    # broadcast; z rows evacuate on DVE off the critical path
                    oq = tmp.tile([32, FREE], f32r, tag="oq", name="oq")
                    nc.vector.tensor_scalar(oq[:], p3[0:32, :], B3[0:32, 0:1], None, Alu.add)
                    pqpool, pqtag = (ps3, "p3") if PS2_4 else (psq, "pq")
                    pq = pqpool.tile([128, FREE], f32, tag=pqtag, name="pq")
                    nc.tensor.matmul(pq[:], BB[0:32, :], oq[:])
                    nc.vector.tensor_scalar(out3s[32:128, :], p3[32:128, :],
                                            B3[32:128, 0:1], None, Alu.add)
                else:
                    # out3s = psum3 + per-partition bias
                    if TAIL_ACT:
                        nc.scalar.activation(out3s[:], p3[:], Act.Identity, bias=B3[:, 0:1])
                    else:
                        nc.vector.tensor_scalar(out3s[:], p3[:], B3[:, 0:1], None, Alu.add)

                    # qrep = broadcast dt*q to component quarters (via PE)
                    pqpool, pqtag = (ps3, "p3") if PS2_4 else (psq, "pq")
                    pq = pqpool.tile([128, FREE], f32, tag=pqtag, name="pq")
                    nc.tensor.matmul(pq[:], BB[:], out3s[:])

                # ---- elementwise state update ----
                if not T3_EARLY and not CHAIN_GP:
                    th = tmp.tile([128, FREE], f32, tag="th", name="th")
                    nc.scalar.activation(th[:], y_pl[:], Act.Tanh)
                    t1 = tmp.tile([128, FREE], f32, tag="t1", name="t1")
                    if T1_ACT:
                        nc.scalar.activation(t1[:], th[:], Act.Copy,
                                             bias=b_coef, scale=a_coef)
                    else:
                        nc.vector.tensor_scalar(t1[:], th[:], a_coef, b_coef,
                                                Alu.mult, Alu.add)
                    t2 = tmp.tile([128, FREE], f32, tag="t2", name="t2")
                    nc.vector.scalar_tensor_tensor(t2[:], t1[:], 1.0, dw_t[:], Alu.mult, Alu.mult)
                    t3 = tmp.tile([128, FREE], f32, tag="t3", name="t3")
                    nc.vector.scalar_tensor_tensor(t3[:], y_pl[:], 1.0 - DT, t2[:], Alu.mult, Alu.add)
                nc.vector.scalar_tensor_tensor(y_pl[:], t3[:], 0.0, pq[:], Alu.add, Alu.add)

                sqA = tmp.tile([32, FREE], f32, tag="sqA", name="sqA")
                p6 = tmp.tile([128, FREE], f32, tag="p6", name="p6")
                if SPLIT_TAIL:
                    nc.gpsimd.tensor_tensor(sqA[:], oq[:], oq[:], Alu.mult)
                    nc.gpsimd.tensor_tensor(accA[:], accA[:], sqA[:], Alu.add)
                    nc.gpsimd.tensor_tensor(p6[32:128, :], out3s[32:128, :],
                                            dw_t[32:128, :], Alu.mult)
                    nc.gpsimd.tensor_tensor(accP[32:128, :], accP[32:128, :],
                                            p6[32:128, :], Alu.add)
                elif GP_ACC:
                    nc.gpsimd.tensor_tensor(sqA[:], out3s[0:32, :], out3s[0:32, :], Alu.mult)
                    nc.gpsimd.tensor_tensor(accA[:], accA[:], sqA[:], Alu.add)
                    nc.gpsimd.tensor_tensor(p6[:], out3s[:], dw_t[:], Alu.mult)
                    nc.gpsimd.tensor_tensor(accP[:], accP[:], p6[:], Alu.add)
                else:
                    nc.scalar.activation(sqA[:], out3s[0:32, :], Act.Square)
                    nc.vector.scalar_tensor_tensor(accA[:], accA[:], 0.0, sqA[:], Alu.add, Alu.add)
                    nc.vector.scalar_tensor_tensor(p6[:], out3s[:], 1.0, dw_t[:], Alu.mult, Alu.mult)
                    nc.vector.scalar_tensor_tensor(accP[:], accP[:], 0.0, p6[:], Alu.add, Alu.add)

            # ---- final loss assembly ----
            ysq = tmp.tile([128, FREE], f32r, tag="ysq", name="ysq")
            nc.scalar.activation(ysq[:], y_pl[:], Act.Square)
            p_term = ps1.tile([32, FREE], f32, tag="p1", name="pterm")
            nc.tensor.matmul(p_term[:], TT[:], ysq[:])
            accPr = tmp.tile([128, FREE], f32r, tag="accPr", name="accPr")
            nc.vector.tensor_copy(accPr[:], accP[:])
            p_P = ps2.tile([32, FREE], f32, tag="p2", name="pP")
            nc.tensor.matmul(p_P[:], TT[:], accPr[:])

            Pg = tmp.tile([32, FREE], f32, tag="Pg", name="Pg")
            nc.vector.tensor_scalar(Pg[:], p_P[:], 0.0, None, Alu.add)
            Tg = tmp.tile([32, FREE], f32, tag="Tg", name="Tg")
            nc.vector.tensor_scalar(Tg[:], p_term[:], 0.0, None, Alu.add)
            D1 = tmp.tile([32, FREE], f32, tag="D1", name="D1")
            nc.vector.scalar_tensor_tensor(D1[:], accA[:], -0.5 / DT, Pg[:], Alu.mult, Alu.add)
            D2 = tmp.tile([32, FREE], f32, tag="D2", name="D2")
            nc.vector.scalar_tensor_tensor(D2[:], Tg[:], -1.0, D1[:], Alu.mult, Alu.add)
            # add Y0 (runtime input, broadcast from SCAL[0])
            sc = consts.tile([4, 1], f32, tag="SCAL", name="SCAL")
            nc.gpsimd.dma_start(sc[:], d_SC)
            y0b = consts.tile([32, 1], f32, tag="y0b", name="y0b")
            nc.gpsimd.dma_start(y0b[:], bass.AP(tensor=d_SC.tensor, offset=0, ap=[[0, 32], [1, 1]]))
            D3 = tmp.tile([32, FREE], f32, tag="D3", name="D3")
            nc.vector.tensor_scalar(D3[:], D2[:], y0b[:, 0:1], None, Alu.add)

            dsq = tmp.tile([32, FREE], f32, tag="dsq", name="dsq")
            res = state.tile([32, 1], f32, tag="res", name="res")
            nc.scalar.activation(dsq[:], D3[:], Act.Square, accum_out=res[:])
            nc.sync.dma_start(d_res, res[:])

    return nc


LAST_EXEC_NS = None
LAST_TRACE_DIR = None


def kernel(**inputs) -> np.ndarray:
    global LAST_EXEC_NS, LAST_TRACE_DIR
    from concourse.bass_utils import run_bass_kernel_spmd
    _install_wait_splitter()
    _install_ldw_opt()

    host = prep_host(inputs)

    nc = build_program()

    scal = np.array([[host["Y0"]], [-0.5 / DT], [0.1 * SQRT_DT], [0.2 * SQRT_DT]], np.float32)
    shared = dict(
        W1S=host["W1S"], CB=host["CB"], W2=host["W2"], B2=host["B2"],
        W3S=host["W3S"], B3=host["B3"], BB=host["BB"], TT=host["TT"],
        YINIT=host["YINIT"], SCAL=scal,
    )
    in_maps = []
    for r in range(N_CORES):
        m = dict(shared)
        m["dWt"] = host["dWt_cores"][r]
        in_maps.append(m)

    trace = os.environ.get("BSDE_TRACE", "0") == "1"
    kw = {}
    if trace:
        kw["trace"] = True
        kw["tmpdir"] = os.environ.get("BSDE_TRACE_DIR") or None
    out = run_bass_kernel_spmd(nc, in_maps, list(range(N_CORES)), **kw)
    LAST_EXEC_NS = getattr(out, "exec_time_ns", None)
    total = np.float64(0.0)
    for r in range(N_CORES):
        total += np.sum(out.results[r]["res"].astype(np.float64))
    return np.float32(total / BATCH)


def _build_in_maps(host):
    scal = np.array([[host["Y0"]], [-0.5 / DT], [0.1 * SQRT_DT], [0.2 * SQRT_DT]], np.float32)
    shared = dict(
        W1S=host["W1S"], CB=host["CB"], W2=host["W2"], B2=host["B2"],
        W3S=host["W3S"], B3=host["B3"], BB=host["BB"], TT=host["TT"],
        YINIT=host["YINIT"], SCAL=scal,
    )
    in_maps = []
    for r in range(N_CORES):
        m = dict(shared)
        m["dWt"] = host["dWt_cores"][r]
        in_maps.append(m)
    return in_maps


def timed_run(nc, in_maps, iters=7):
    """Mirror bass2jax.run_bass_via_pjrt's multi-core path, but keep inputs
    device-resident and time steady-state executions. Returns (results_core0,
    sorted wall times in ns per call)."""
    import time
    import jax
    from jax.sharding import Mesh, PartitionSpec, NamedSharding
    from jax.experimental.shard_map import shard_map
    from concourse import bass2jax, mybir

    bass2jax.install_neuronx_cc_hook()
    n_cores = N_CORES

    in_names, out_names, out_avals, zero_outs = [], [], [], []
    for alloc in nc.m.functions[0].allocations:
        if not isinstance(alloc, mybir.MemoryLocationSet):
            continue
        name = alloc.memorylocations[0].name
        if alloc.kind == "ExternalInput":
            in_names.append(name)
        elif alloc.kind == "ExternalOutput":
            out_names.append(name)
            shape = tuple(alloc.tensor_shape)
            dtype = mybir.dt.np(alloc.dtype)
            out_avals.append(jax.core.ShapedArray(shape, dtype))
            zero_outs.append(np.zeros(shape, dtype))
    n_params = len(in_names)
    n_outs = len(out_avals)
    all_names = in_names + out_names
    donate = tuple(range(n_params, n_params + n_outs))

    def _body(*args):
        outs = bass2jax._bass_exec_p.bind(
            *list(args),
            out_avals=tuple(out_avals),
            in_names=tuple(all_names),
            out_names=tuple(out_names),
            lowering_input_output_aliases=(),
            sim_require_finite=True,
            sim_require_nnan=True,
            nc=nc,
        )
        return tuple(outs)

    devices = jax.devices()[:n_cores]
    mesh = Mesh(np.asarray(devices), ("core",))
    in_specs = (PartitionSpec("core"),) * (n_params + n_outs)
    out_specs = (PartitionSpec("core"),) * len(out_names)
    sharded = jax.jit(
        shard_map(_body, mesh=mesh, in_specs=in_specs, out_specs=out_specs, check_rep=False),
        donate_argnums=donate,
        keep_unused=True,
    )
    concat_in = [
        np.concatenate([np.asarray(in_maps[c][nm]) for c in range(n_cores)], axis=0)
        for nm in in_names
    ]
    sh = NamedSharding(mesh, PartitionSpec("core"))
    dev_in = [jax.device_put(a, sh) for a in concat_in]
    concat_zeros = [np.zeros((n_cores * z.shape[0], *z.shape[1:]), z.dtype) for z in zero_outs]

    out = sharded(*dev_in, *concat_zeros)   # warm-up / compile
    jax.block_until_ready(out)
    times = []
    for _ in range(iters):
        zz = [np.zeros((n_cores * z.shape[0], *z.shape[1:]), z.dtype) for z in zero_outs]
        t0 = time.perf_counter_ns()
        out = sharded(*dev_in, *zz)
        jax.block_until_ready(out)
        times.append(time.perf_counter_ns() - t0)
    res0 = {
        nm: np.asarray(out[i]).reshape(n_cores, *out_avals[i].shape)
        for i, nm in enumerate(out_names)
    }
    return res0, sorted(times)


if __name__ == "__main__":
    rng = np.random.default_rng(0)
    fake = {
        "y0": rng.standard_normal(3).astype(np.float32),
        "Y0": np.zeros((1, 1), np.float32),
        "qW1": rng.standard_normal((4, 64)).astype(np.float32) * 0.5,
        "qb1": np.zeros(64, np.float32),
        "qW2": rng.standard_normal((64, 64)).astype(np.float32) * 0.12,
        "qb2": np.zeros(64, np.float32),
        "qW3": rng.standard_normal((64, 1)).astype(np.float32) * 0.12,
        "qb3": np.zeros(1, np.float32),
        "zW1": rng.standard_normal((4, 64)).astype(np.float32) * 0.5,
        "zb1": np.zeros(64, np.float32),
        "zW2": rng.standard_normal((64, 64)).astype(np.float32) * 0.12,
        "zb2": np.zeros(64, np.float32),
        "zW3": rng.standard_normal((64, 3)).astype(np.float32) * 0.12,
        "zb3": np.zeros(3, np.float32),
        "dW": rng.standard_normal((N_STEPS, BATCH, 3)).astype(np.float32),
    }
    print(kernel(**fake))

